# revision 61
# baseline (speedup 1.0000x reference)
"""Trainium2 Bass kernel for single-head attention with RoPE.

Problem (per full input): x [256, 200, 1024], wq/wk/wv [128, 1024], wo [1024, 128]
  q/k/v = x @ w*.T ; RoPE on q,k (positions 1..S-1, class token 0 unrotated)
  out = softmax(q k^T / sqrt(128)) v @ wo.T

Strategy: data-parallel over batch across 8 NeuronCores (32 batches/core),
processed in 16 blocks of 2 batches (400 tokens). All I/O is half-width:
x ships as an error-compensated fp8 hi/lo pair, the output returns fp16.

Per block:
  - QKV projections run on the PE in fp8e4 DoubleRow mode (K=256 per pass).
    Plain fp8 is too coarse (~6% per-element), so operands are split
    x ~ x_hi + x_lo, w ~ w_hi + w_lo (both fp8; w pre-scaled by 64 and x by 8
    so the residuals stay in fp8's normal range) and three of the four
    products are accumulated in PSUM: hi*hi (4 DR matmuls over d-chunk pairs)
    plus the two cross terms, computed 2-at-a-time by pairing DoubleRow's two
    k-tiles as (x_hi, x_lo) against (w_lo, w_hi) per d-chunk (8 DR matmuls).
    The dropped lo*lo term is ~0.4%% of the result. The 1/512 prescale comes
    out in the PSUM->SBUF evacuation (free ACT constant scale).
  - RoPE in de-interleaved rotate-half form (wq/wk rows permuted host-side):
    the half-swap runs as a fp16 permutation matmul on the PE; the two
    multiplies and the add are spread over DVE and GPSIMD with fp16 tables.
  - scores [k, q] only (k on partitions): softmax denominators come from
    tiny ones-vector matmuls (est^T @ 1 -> [q,1] in PSUM) instead of a second
    transposed score pass; exp on ACT writes fp16 est straight to SBUF.
  - V is transposed to seq-major via fp16 PE transposes (fp16 PSUM out).
  - output projection per 128-token chunk (chunks span the two batches);
    softmax normalization (1/rowsum) folds into the PSUM->SBUF evacuation as
    a per-partition scale, which also converts to fp16 for the store. The
    ragged 16-token tail of each block is parked at partition slot 64*(blk%2)
    (only bases 0/32/64 are legal matmul output positions) and two blocks'
    tails share one outproj chunk, quartering the tail cost.
  - stores go out per chunk into a [blk, p, chunk] layout (plus the tail
    tensor); the host reassembles and upcasts. PSUM's 8 banks are split into
    four 2-buffer pools by use class so next-block projections only wait on
    this block's evacuations; the last block rotates its outproj through the
    idle pools to drain faster.
"""

import math

import numpy as np
import ml_dtypes

import concourse.bass as bass
import concourse.mybir as mybir
import concourse.tile as tile
from concourse.bass_utils import run_bass_kernel_spmd

B, S, DIM, HD = 256, 200, 1024, 128
BASE = 10000.0
N_CORES = 8
BS = B // N_CORES      # 32 batches per core
BB = 2                 # batches per block
TB = BB * S            # 400 tokens per block
NBLK = BS // BB        # 16 blocks per core
NDC = DIM // 128       # 8 contraction chunks
F32 = mybir.dt.float32
F16 = mybir.dt.float16
FP8 = mybir.dt.float8e4
DRMODE = mybir.MatmulPerfMode.DoubleRow
EXP_SCALE = 1.0 / math.sqrt(HD)
XS = 8.0               # x fp8 prescale
WS = 64.0              # weight fp8 prescale
EVAC_SCALE = 1.0 / (XS * WS)
# output-token chunks within a block (span the batch boundary; tokens are
# contiguous in [BS*S, DIM] so each chunk stores as one 2D DMA)
TCH = [(0, 128), (128, 128), (256, 128), (384, 16)]
# scores/est partition chunks over k positions within one batch
KCH = [(0, 128), (128, S - 128)]


class _TileContextSplitDrain(tile.TileContext):
    """Workaround: this walrus build rejects >2 sem-wait commands on the
    kernel-tail Drain. Emit each needed wait as its own instruction first."""

    def _drain_and_barrier(self, tick_clock, wait_clock):
        nc = self.nc
        fake = mybir.InstNoOp(
            name=nc.get_next_instruction_name(), ins=[], outs=[],
            engine=mybir.EngineType.SP,
        )
        wait_clock.add_sem_waits(
            fake, tile.ScopedClock({None: tick_clock.global_clock})
        )
        waits = list(fake.sync_info.on_wait) if fake.sync_info is not None else []
        assert self.sems is not None
        handles = {h.name: h for h in self.sems.allocated().values()}
        for w in waits:
            nc.sync.wait_ge(handles[w.ant_name], w.wait_value)
        nc.sync.drain()
        nc.all_engine_barrier()
        popped = nc._tile_sem_poison_stack.pop()
        assert popped is self._sem_poison
        nc.clear_and_free_semaphores(list(self.sems.allocated().values()))
        nc.all_engine_barrier()


def _split_excess_waits(nc):
    """This walrus build accepts 1 sem-wait per instruction (2 on
    EventSemaphore). Tile may attach more; hoist the excess onto standalone
    EventSemaphore instructions right before the owner (same engine, so
    in-order issue preserves the wait semantics)."""
    n = 0
    for b in nc.m.functions[0].blocks:
        insts = b.instructions
        out = []
        for i in insts:
            si = i.sync_info
            if si is not None and len(si.on_wait) > 1:
                keep = 2 if isinstance(i, mybir.InstEventSemaphore) else 1
                waits = list(si.on_wait)
                for w in waits[:-keep] if keep < len(waits) else []:
                    n += 1
                    out.append(mybir.InstEventSemaphore(
                        name=f"{i.name}-evw{n}", ins=[], outs=[],
                        engine=i.engine,
                        sync_info=mybir.SyncInfo(on_wait=[w], on_update=[]),
                    ))
                i.sync_info = mybir.SyncInfo(
                    on_wait=waits[-keep:], on_update=list(si.on_update)
                )
            out.append(i)
        b.instructions = out
    return n


def _build_nc():
    nc = bass.Bass("TRN2", target_bir_lowering=False, debug=False)

    # x hi/lo interleaved: [:, :, dc, 0, :] = hi, [:, :, dc, 1, :] = lo
    xt8 = nc.dram_tensor("xt8", [NBLK, 128, NDC, 2, TB], FP8,
                         kind="ExternalInput").ap()
    # weights (lo, hi) order so a (x_hi, x_lo) rhs pairing yields cross terms
    wq8 = nc.dram_tensor("wq8", [128, NDC, 2, HD], FP8, kind="ExternalInput").ap()
    wk8 = nc.dram_tensor("wk8", [128, NDC, 2, HD], FP8, kind="ExternalInput").ap()
    wv8 = nc.dram_tensor("wv8", [128, NDC, 2, HD], FP8, kind="ExternalInput").ap()
    wot = nc.dram_tensor("wot", [HD, DIM], F16, kind="ExternalInput").ap()
    cosf = nc.dram_tensor("cosf", [128, TB], F16, kind="ExternalInput").ap()
    sinf = nc.dram_tensor("sinf", [128, TB], F16, kind="ExternalInput").ap()
    ident = nc.dram_tensor("ident", [128, 128], F16, kind="ExternalInput").ap()
    p64 = nc.dram_tensor("p64", [128, 128], F16, kind="ExternalInput").ap()
    ones = nc.dram_tensor("ones", [128, 1], F16, kind="ExternalInput").ap()
    # big store: token (blk, c*128 + p) lives at out0[blk, p, c]; the ragged
    # 16-token tail of each block goes to out1. Host reassembles.
    out0 = nc.dram_tensor("out0", [NBLK, 128, 3 * DIM], F16,
                          kind="ExternalOutput").ap()
    out1 = nc.dram_tensor("out1", [NBLK // 2, 2, 16, DIM], F16,
                          kind="ExternalOutput").ap()

    with _TileContextSplitDrain(nc) as tc:
        with (
            tc.tile_pool(name="singles", bufs=1) as singles,
            tc.tile_pool(name="xt", bufs=4) as xt_pool,
            # PSUM is 8 banks; four 2-buf pools keyed by use class so that
            # next-block projections only wait on this block's evacuations,
            # not on the whole attention chain.
            tc.tile_pool(name="qkv_ps", bufs=2, space="PSUM") as qkv_ps,
            tc.tile_pool(name="swsc_ps", bufs=2, space="PSUM") as swsc_ps,
            tc.tile_pool(name="misc_ps", bufs=2, space="PSUM") as misc_ps,
            tc.tile_pool(name="out_ps", bufs=2, space="PSUM") as out_ps,
            tc.tile_pool(name="ropetmp", bufs=6) as ropetmp,
            tc.tile_pool(name="heads", bufs=6) as heads,
            tc.tile_pool(name="attn_sb", bufs=6) as attn_sb_pool,
            tc.tile_pool(name="stats", bufs=12) as stats,
            tc.tile_pool(name="outsb", bufs=6) as outsb,
        ):
            # ---- PE p-state pre-warm: the first real matmul cannot start
            # until the weight/x DMA latency chain (~3.5us) resolves, and
            # the PE clock needs ~3us of continuous busy to reach 2.4GHz.
            # Chew through dummy matmuls on a memset tile meanwhile so real
            # work starts at full clock. ----
            warm = singles.tile([128, 256], F16, name="warm", tag="warm")
            nc.gpsimd.memset(warm, 0.0)
            for i in range(16):
                wps = out_ps.tile([128, 256], F32, name="warm_ps",
                                  tag="out_ps")
                nc.tensor.matmul(wps, lhsT=warm[:, 0:128], rhs=warm,
                                 start=True, stop=True)

            # ---- one-time loads (wk + first x chunks first: the k
            # projection is issued first and gates block 0) ----
            w_sb = {}
            t = singles.tile([128, NDC, 2, HD], FP8, name="wk8", tag="wk8")
            nc.sync.dma_start(out=t, in_=wk8)
            w_sb["k"] = t
            xt0_sb = xt_pool.tile([128, NDC, 2, TB], FP8, name="xt", tag="xt")
            for j in range(4):
                nc.sync.dma_start(out=xt0_sb[:, 2 * j:2 * j + 2, :, :],
                                  in_=xt8[0, :, 2 * j:2 * j + 2, :, :])
            for name, src in (("q", wq8), ("v", wv8)):
                t = singles.tile([128, NDC, 2, HD], FP8, name="w" + name,
                                 tag="w" + name)
                nc.sync.dma_start(out=t, in_=src)
                w_sb[name] = t
            p64_sb = singles.tile([128, 128], F16, name="p64", tag="p64")
            nc.sync.dma_start(out=p64_sb, in_=p64)
            cos_sb = singles.tile([128, TB], F16, name="cosf", tag="cosf")
            nc.sync.dma_start(out=cos_sb, in_=cosf)
            sin_sb = singles.tile([128, TB], F16, name="sinf", tag="sinf")
            nc.sync.dma_start(out=sin_sb, in_=sinf)
            id_sb = singles.tile([128, 128], F16, name="ident", tag="ident")
            nc.sync.dma_start(out=id_sb, in_=ident)
            ones_sb = singles.tile([128, 1], F16, name="ones", tag="ones")
            nc.sync.dma_start(out=ones_sb, in_=ones)
            wot_sb = singles.tile([HD, DIM], F16, name="wot", tag="wot")
            nc.sync.dma_start(out=wot_sb, in_=wot)

            for blk in range(NBLK):
                # ---- load x hi/lo for this block ----
                if blk == 0:
                    xt_sb = xt0_sb
                else:
                    xt_sb = xt_pool.tile([128, NDC, 2, TB], FP8, name="xt",
                                         tag="xt")
                    nc.sync.dma_start(out=xt_sb[:, 0:4, :, :],
                                      in_=xt8[blk, :, 0:4, :, :])
                    nc.sync.dma_start(out=xt_sb[:, 4:8, :, :],
                                      in_=xt8[blk, :, 4:8, :, :])

                # ---- QKV projections: fp8 DoubleRow, hi/lo compensated ----
                def proj(wname):
                    w = w_sb[wname]
                    ps = qkv_ps.tile([128, TB], F32, name="proj_ps",
                                     tag="proj_ps")
                    mms = []
                    for j in range(NDC // 2):   # hi @ hi over d-chunk pairs
                        mms.append((w[:, 2 * j:2 * j + 2, 1, :],
                                    xt_sb[:, 2 * j:2 * j + 2, 0, :]))
                    for dc in range(NDC):       # x_hi@w_lo + x_lo@w_hi
                        mms.append((w[:, dc, :, :], xt_sb[:, dc, :, :]))
                    for i, (lhs, rhs) in enumerate(mms):
                        nc.tensor.matmul(ps, lhsT=lhs, rhs=rhs,
                                         start=(i == 0),
                                         stop=(i == len(mms) - 1),
                                         perf_mode=DRMODE)
                    return ps

                # k first: its rope chain gates the score matmuls
                k_ps = proj("k")
                q_ps = proj("q")
                v_ps = proj("v")

                # ---- RoPE (de-interleaved rotate-half form) ----
                # swap(q)[p] = q[(p+64)%128] runs on the PE via a permutation
                # matmul (rhs must be SBUF, hence the ACT evacuation first,
                # which also removes the 512x fp8 prescale).
                def rope(ps, tag, fast):
                    # q (the late chain, gating scores) runs split per batch
                    # half with the final add on DVE right behind the u-mul
                    # (same in-order queue, no sem hop). k has timeline
                    # slack and keeps the cheap Pool path.
                    qsb = ropetmp.tile([128, TB], F16, name="pre_" + tag,
                                       tag="pre_" + tag)
                    sw_ps = swsc_ps.tile([128, TB], F32, name="swsc_ps",
                                         tag="swsc_ps")
                    c = ropetmp.tile([128, TB], F16, name="rope_c" + tag,
                                     tag="rope_c" + tag)
                    u = ropetmp.tile([128, TB], F16, name="rope_u" + tag,
                                     tag="rope_u" + tag)
                    h = heads.tile([128, TB], F16, name=tag, tag=tag)
                    sls = ([slice(i * S, (i + 1) * S) for i in range(BB)]
                           if fast else [slice(0, TB)])
                    # latency-critical chain: bias the scheduler to pick
                    # these over the previous block's throughput work
                    with tc.high_priority(offset=250):
                        for sl in sls:
                            nc.scalar.mul(qsb[:, sl], ps[:, sl], EVAC_SCALE)
                            nc.tensor.matmul(sw_ps[:, sl], lhsT=p64_sb,
                                             rhs=qsb[:, sl],
                                             start=True, stop=True)
                            nc.gpsimd.tensor_mul(c[:, sl], qsb[:, sl],
                                                 cos_sb[:, sl])
                            # sin table is sign-folded ([-sin; +sin]) so one
                            # add completes the rotation
                            nc.vector.tensor_mul(u[:, sl], sw_ps[:, sl],
                                                 sin_sb[:, sl])
                            if fast:
                                nc.vector.tensor_add(h[:, sl], c[:, sl],
                                                     u[:, sl])
                            else:
                                nc.gpsimd.tensor_add(h[:, sl], c[:, sl],
                                                     u[:, sl])
                    return h

                k_h = rope(k_ps, "k_h", fast=False)
                q_h = rope(q_ps, "q_h", fast=True)
                v_h = heads.tile([128, TB], F16, name="v_h", tag="v_h")
                nc.scalar.mul(v_h, v_ps, EVAC_SCALE)

                # ---- scores [k, q] and exp ----
                est = []
                for kc, (k0, ksz) in enumerate(KCH):
                    sp = swsc_ps.tile([128, TB], F32, name="swsc_ps",
                                      tag="swsc_ps")
                    for i in range(BB):
                        nc.tensor.matmul(
                            sp[0:ksz, i * S:(i + 1) * S],
                            lhsT=k_h[:, i * S + k0: i * S + k0 + ksz],
                            rhs=q_h[:, i * S:(i + 1) * S],
                            start=True, stop=True,
                        )
                    e = attn_sb_pool.tile([128, TB], F16, name="exp_st",
                                          tag="exp_st")
                    hp = tc.high_priority(offset=250)
                    hp.__enter__()
                    if blk == NBLK - 1:
                        for i in range(BB):
                            nc.scalar.activation(
                                out=e[0:ksz, i * S:(i + 1) * S],
                                in_=sp[0:ksz, i * S:(i + 1) * S],
                                func=mybir.ActivationFunctionType.Exp,
                                scale=EXP_SCALE,
                            )
                    else:
                        nc.scalar.activation(
                            out=e[0:ksz, :], in_=sp[0:ksz, :],
                            func=mybir.ActivationFunctionType.Exp,
                            scale=EXP_SCALE,
                        )
                    hp.__exit__(None, None, None)
                    est.append(e)

                # ---- softmax denominators: est^T @ 1 per token chunk ----
                # tail (16 tokens) goes to partition slot 64*(blk%2) so two
                # blocks' tails share one 128-partition group chunk later
                # (only bases 0/32/64 are legal; quadrant 3 is unusable).
                gslot = 64 * (blk % 2)
                if blk % 2 == 0:
                    grp_rec = stats.tile([128, 1], F32, name="grec",
                                         tag="grec")
                    nc.gpsimd.memset(grp_rec, 1.0)
                    grp_av = attn_sb_pool.tile([128, 128], F16, name="gav",
                                               tag="gav")
                    nc.gpsimd.memset(grp_av, 0.0)
                recips = {}
                sums_ps = misc_ps.tile([128, 8], F32, name="misc_ps",
                                       tag="misc_ps")
                for tc_i, (t0, tsz) in enumerate(TCH):
                    p0 = gslot if tc_i == 3 else 0
                    for kc, (k0, ksz) in enumerate(KCH):
                        nc.tensor.matmul(
                            sums_ps[p0:p0 + tsz, tc_i:tc_i + 1],
                            lhsT=est[kc][0:ksz, t0:t0 + tsz],
                            rhs=ones_sb[0:ksz, :],
                            start=(kc == 0), stop=(kc == len(KCH) - 1),
                        )
                    if tc_i == 3:
                        nc.vector.reciprocal(
                            grp_rec[p0:p0 + tsz, :],
                            sums_ps[p0:p0 + tsz, tc_i:tc_i + 1])
                    else:
                        rec = stats.tile([128, 1], F32, name=f"recip{tc_i}",
                                         tag=f"recip{tc_i}")
                        nc.vector.reciprocal(rec[0:tsz, :],
                                             sums_ps[0:tsz, tc_i:tc_i + 1])
                        recips[tc_i] = rec

                # ---- V -> seq-major via fp16 PE transpose, per batch ----
                vt_ps = misc_ps.tile([128, 512], F16, name="misc_ps",
                                     tag="misc_ps")
                vt_sbs = []
                for i in range(BB):
                    nc.tensor.transpose(
                        vt_ps[0:128, i * 256: i * 256 + 128],
                        v_h[:, i * S: i * S + 128], id_sb,
                    )
                    nc.tensor.transpose(
                        vt_ps[0:72, i * 256 + 128: i * 256 + 256],
                        v_h[:, i * S + 128: (i + 1) * S], id_sb,
                    )
                    vt_sb = attn_sb_pool.tile([128, 256], F16, name="vt_sb",
                                              tag="vt_sb")
                    nc.vector.tensor_copy(
                        vt_sb[0:128, 0:128],
                        vt_ps[0:128, i * 256: i * 256 + 128])
                    nc.vector.tensor_copy(
                        vt_sb[0:72, 128:256],
                        vt_ps[0:72, i * 256 + 128: i * 256 + 256])
                    vt_sbs.append(vt_sb)

                # ---- AV: attn_head[h, q] (unnormalized) ----
                av_ps = misc_ps.tile([128, TB], F32, name="misc_ps",
                                     tag="misc_ps")
                for i in range(BB):
                    nc.tensor.matmul(
                        av_ps[:, i * S:(i + 1) * S],
                        lhsT=vt_sbs[i][0:128, 0:128],
                        rhs=est[0][0:128, i * S:(i + 1) * S],
                        start=True, stop=False,
                    )
                    nc.tensor.matmul(
                        av_ps[:, i * S:(i + 1) * S],
                        lhsT=vt_sbs[i][0:72, 128:256],
                        rhs=est[1][0:72, i * S:(i + 1) * S],
                        start=False, stop=True,
                    )
                av_sb = attn_sb_pool.tile([128, TB], F16, name="av_sb",
                                          tag="av_sb")
                # two halves so the first outproj chunks start earlier;
                # the 16-token tail goes to this group's shared tile
                if blk == NBLK - 1:
                    nc.vector.tensor_copy(av_sb[:, 0:128], av_ps[:, 0:128])
                    nc.vector.tensor_copy(av_sb[:, 128:256],
                                          av_ps[:, 128:256])
                    nc.vector.tensor_copy(av_sb[:, 256:384],
                                          av_ps[:, 256:384])
                else:
                    nc.vector.tensor_copy(av_sb[:, 0:200], av_ps[:, 0:200])
                    nc.vector.tensor_copy(av_sb[:, 200:384],
                                          av_ps[:, 200:384])
                nc.vector.tensor_copy(grp_av[:, gslot:gslot + 16],
                                      av_ps[:, 384:TB])

                # ---- output projection + normalization + store ----
                # full 128-token chunks span the batch boundary; stores go
                # out per chunk so they pipeline with the evacuations. The
                # last block borrows the (by then idle) other PSUM pools for
                # a deeper outproj rotation, since there is no next-block
                # work left to hide the evacuation latency behind.
                last = blk == NBLK - 1
                osb = outsb.tile([128, 3 * DIM], F16, name="osb", tag="osb")
                for tc_i, (t0, tsz) in enumerate(TCH[:3]):
                    for dc in range(2):
                        pool, tag = ([(out_ps, "out_ps"), (misc_ps, "misc_ps"),
                                      (swsc_ps, "swsc_ps")][(tc_i * 2 + dc) % 3]
                                     if last else (out_ps, "out_ps"))
                        ops = pool.tile([128, 512], F32, name=tag, tag=tag)
                        nc.tensor.matmul(
                            ops[0:tsz, :],
                            lhsT=av_sb[:, t0:t0 + tsz],
                            rhs=wot_sb[:, dc * 512:(dc + 1) * 512],
                            start=True, stop=True,
                        )
                        dst = osb[0:tsz,
                                  tc_i * DIM + dc * 512:
                                  tc_i * DIM + (dc + 1) * 512]
                        if dc == 0:
                            nc.scalar.mul(dst, ops[0:tsz, :],
                                          recips[tc_i][0:tsz, :])
                        else:
                            nc.vector.tensor_scalar_mul(
                                dst, ops[0:tsz, :], recips[tc_i][0:tsz, :])
                    nc.sync.dma_start(
                        out=out0[blk, :, tc_i * DIM:(tc_i + 1) * DIM],
                        in_=osb[:, tc_i * DIM:(tc_i + 1) * DIM])

                # ---- grouped tail outproj every 4th block ----
                if blk % 2 == 1:
                    gosb = outsb.tile([128, DIM], F16, name="gosb",
                                      tag="gosb")
                    for dc in range(2):
                        ops = out_ps.tile([128, 512], F32, name="out_ps",
                                          tag="out_ps")
                        nc.tensor.matmul(
                            ops,
                            lhsT=grp_av,
                            rhs=wot_sb[:, dc * 512:(dc + 1) * 512],
                            start=True, stop=True,
                        )
                        dst = gosb[:, dc * 512:(dc + 1) * 512]
                        if dc == 0:
                            nc.scalar.mul(dst, ops, grp_rec)
                        else:
                            nc.vector.tensor_scalar_mul(dst, ops, grp_rec)
                    nc.sync.dma_start(out=out1[blk // 2, 0],
                                      in_=gosb[0:16, :])
                    nc.sync.dma_start(out=out1[blk // 2, 1],
                                      in_=gosb[64:80, :])
    _split_excess_waits(nc)
    return nc


_NC_CACHE = {}


def _get_nc():
    if "nc" not in _NC_CACHE:
        _NC_CACHE["nc"] = _build_nc()
    return _NC_CACHE["nc"]


def _hilo(a):
    hi = np.asarray(a, ml_dtypes.float8_e4m3fn)
    lo = np.asarray(a - hi.astype(np.float32), ml_dtypes.float8_e4m3fn)
    return hi, lo


def _host_prep(x, wq, wk, wv, wo):
    """Shared (non-x) device inputs + per-core x fp8 hi/lo shards."""
    perm = np.concatenate([np.arange(0, HD, 2), np.arange(1, HD, 2)])

    def wprep(w, permute):
        wp = (w[perm] if permute else w) * WS
        # layout [p, dc, 2, h]: row d of w.T at (p=d%128, dc=d//128);
        # index 2 is (lo, hi)
        wt = np.ascontiguousarray(
            wp.T.reshape(NDC, 128, HD).transpose(1, 0, 2))
        hi, lo = _hilo(wt)
        return np.ascontiguousarray(np.stack([lo, hi], axis=2))

    wq8 = wprep(wq, True)
    wk8 = wprep(wk, True)
    wv8 = wprep(wv, False)
    wot = np.ascontiguousarray(wo.T.astype(np.float16))

    inv_freq = 1.0 / BASE ** (np.arange(0, HD, 2, dtype=np.float64) / HD)
    ang = np.zeros((S, HD // 2), np.float64)
    ang[1:] = np.arange(S - 1, dtype=np.float64)[:, None] * inv_freq[None, :]
    cos_t = np.cos(ang).T  # [64, S]
    sin_t = np.sin(ang).T
    cosf = np.tile(np.concatenate([cos_t, cos_t], axis=0),
                   (1, BB)).astype(np.float16)
    # sign-folded: rotated = q*cosf + swap64(q)*sinf in one add
    sinf = np.tile(np.concatenate([-sin_t, sin_t], axis=0),
                   (1, BB)).astype(np.float16)

    shared = {
        "wq8": wq8, "wk8": wk8, "wv8": wv8, "wot": wot,
        "cosf": np.ascontiguousarray(cosf),
        "sinf": np.ascontiguousarray(sinf),
        "ident": np.eye(128, dtype=np.float16),
        "p64": np.ascontiguousarray(
            np.roll(np.eye(128, dtype=np.float16), 64, axis=1)),
        "ones": np.ones((128, 1), np.float16),
    }
    xs = x.reshape(N_CORES, NBLK, TB, NDC, 128) * XS
    # [core, blk, p, dc, t]
    xt = np.ascontiguousarray(xs.transpose(0, 1, 4, 3, 2))
    hi, lo = _hilo(xt)
    x8 = np.stack([hi, lo], axis=4)  # [core, blk, p, dc, 2, t]
    xts = [np.ascontiguousarray(x8[c]) for c in range(N_CORES)]
    return shared, xts


def kernel(x, wq, wk, wv, wo):
    x = np.asarray(x, np.float32)
    wq = np.asarray(wq, np.float32)
    wk = np.asarray(wk, np.float32)
    wv = np.asarray(wv, np.float32)
    wo = np.asarray(wo, np.float32)

    shared, xts = _host_prep(x, wq, wk, wv, wo)
    in_maps = [dict(shared, xt8=xts[c]) for c in range(N_CORES)]
    nc = _get_nc()
    res = run_bass_kernel_spmd(nc, in_maps, list(range(N_CORES)))
    full = np.empty((B * S, DIM), np.float32)
    for c in range(N_CORES):
        o0 = np.asarray(res.results[c]["out0"]).astype(np.float32)
        o1 = np.asarray(res.results[c]["out1"]).astype(np.float32)
        core = full[c * BS * S:(c + 1) * BS * S].reshape(NBLK, TB, DIM)
        # token (blk, cch*128 + p) came from out0[blk, p, cch]
        core[:, 0:384, :] = o0.reshape(NBLK, 128, 3, DIM).transpose(
            0, 2, 1, 3).reshape(NBLK, 384, DIM)
        # tail token (blk, 384+i) came from out1[blk//2, 64*(blk%2)+i]
        core[:, 384:400, :] = o1.reshape(NBLK, 16, DIM)
    return full.reshape(B, S, DIM)


# revision 62
# speedup vs baseline: 1.0201x; 1.0201x over previous
"""Trainium2 Bass kernel for single-head attention with RoPE.

Problem (per full input): x [256, 200, 1024], wq/wk/wv [128, 1024], wo [1024, 128]
  q/k/v = x @ w*.T ; RoPE on q,k (positions 1..S-1, class token 0 unrotated)
  out = softmax(q k^T / sqrt(128)) v @ wo.T

Strategy: data-parallel over batch across 8 NeuronCores (32 batches/core),
processed in 16 blocks of 2 batches (400 tokens). All I/O is half-width:
x ships as an error-compensated fp8 hi/lo pair, the output returns fp16.

Per block:
  - QKV projections run on the PE in fp8e4 DoubleRow mode (K=256 per pass).
    Plain fp8 is too coarse (~6% per-element), so operands are split
    x ~ x_hi + x_lo, w ~ w_hi + w_lo (both fp8; w pre-scaled by 64 and x by 8
    so the residuals stay in fp8's normal range) and three of the four
    products are accumulated in PSUM: hi*hi (4 DR matmuls over d-chunk pairs)
    plus the two cross terms, computed 2-at-a-time by pairing DoubleRow's two
    k-tiles as (x_hi, x_lo) against (w_lo, w_hi) per d-chunk (8 DR matmuls).
    The dropped lo*lo term is ~0.4%% of the result. The 1/512 prescale comes
    out in the PSUM->SBUF evacuation (free ACT constant scale).
  - RoPE in de-interleaved rotate-half form (wq/wk rows permuted host-side):
    the half-swap runs as a fp16 permutation matmul on the PE; the two
    multiplies and the add are spread over DVE and GPSIMD with fp16 tables.
  - scores [k, q] only (k on partitions): softmax denominators come from
    tiny ones-vector matmuls (est^T @ 1 -> [q,1] in PSUM) instead of a second
    transposed score pass; exp on ACT writes fp16 est straight to SBUF.
  - V is transposed to seq-major via fp16 PE transposes (fp16 PSUM out).
  - output projection per 128-token chunk (chunks span the two batches);
    softmax normalization (1/rowsum) folds into the PSUM->SBUF evacuation as
    a per-partition scale, which also converts to fp16 for the store. The
    ragged 16-token tail of each block is parked at partition slot 64*(blk%2)
    (only bases 0/32/64 are legal matmul output positions) and two blocks'
    tails share one outproj chunk, quartering the tail cost.
  - stores go out per chunk into a [blk, p, chunk] layout (plus the tail
    tensor); the host reassembles and upcasts. PSUM's 8 banks are split into
    four 2-buffer pools by use class so next-block projections only wait on
    this block's evacuations; the last block rotates its outproj through the
    idle pools to drain faster.
"""

import math

import numpy as np
import ml_dtypes

import concourse.bass as bass
import concourse.mybir as mybir
import concourse.tile as tile
from concourse.bass_utils import run_bass_kernel_spmd

B, S, DIM, HD = 256, 200, 1024, 128
BASE = 10000.0
N_CORES = 8
BS = B // N_CORES      # 32 batches per core
BB = 2                 # batches per block
TB = BB * S            # 400 tokens per block
NBLK = BS // BB        # 16 blocks per core
NDC = DIM // 128       # 8 contraction chunks
F32 = mybir.dt.float32
F16 = mybir.dt.float16
FP8 = mybir.dt.float8e4
DRMODE = mybir.MatmulPerfMode.DoubleRow
EXP_SCALE = 1.0 / math.sqrt(HD)
XS = 8.0               # x fp8 prescale
WS = 64.0              # weight fp8 prescale
EVAC_SCALE = 1.0 / (XS * WS)
# output-token chunks within a block (span the batch boundary; tokens are
# contiguous in [BS*S, DIM] so each chunk stores as one 2D DMA)
TCH = [(0, 128), (128, 128), (256, 128), (384, 16)]
# scores/est partition chunks over k positions within one batch
KCH = [(0, 128), (128, S - 128)]


class _TileContextSplitDrain(tile.TileContext):
    """Workaround: this walrus build rejects >2 sem-wait commands on the
    kernel-tail Drain. Emit each needed wait as its own instruction first."""

    def _drain_and_barrier(self, tick_clock, wait_clock):
        nc = self.nc
        fake = mybir.InstNoOp(
            name=nc.get_next_instruction_name(), ins=[], outs=[],
            engine=mybir.EngineType.SP,
        )
        wait_clock.add_sem_waits(
            fake, tile.ScopedClock({None: tick_clock.global_clock})
        )
        waits = list(fake.sync_info.on_wait) if fake.sync_info is not None else []
        assert self.sems is not None
        handles = {h.name: h for h in self.sems.allocated().values()}
        for w in waits:
            nc.sync.wait_ge(handles[w.ant_name], w.wait_value)
        nc.sync.drain()
        nc.all_engine_barrier()
        popped = nc._tile_sem_poison_stack.pop()
        assert popped is self._sem_poison
        nc.clear_and_free_semaphores(list(self.sems.allocated().values()))
        nc.all_engine_barrier()


def _split_excess_waits(nc):
    """This walrus build accepts 1 sem-wait per instruction (2 on
    EventSemaphore). Tile may attach more; hoist the excess onto standalone
    EventSemaphore instructions right before the owner (same engine, so
    in-order issue preserves the wait semantics)."""
    n = 0
    for b in nc.m.functions[0].blocks:
        insts = b.instructions
        out = []
        for i in insts:
            si = i.sync_info
            if si is not None and len(si.on_wait) > 1:
                keep = 2 if isinstance(i, mybir.InstEventSemaphore) else 1
                waits = list(si.on_wait)
                for w in waits[:-keep] if keep < len(waits) else []:
                    n += 1
                    out.append(mybir.InstEventSemaphore(
                        name=f"{i.name}-evw{n}", ins=[], outs=[],
                        engine=i.engine,
                        sync_info=mybir.SyncInfo(on_wait=[w], on_update=[]),
                    ))
                i.sync_info = mybir.SyncInfo(
                    on_wait=waits[-keep:], on_update=list(si.on_update)
                )
            out.append(i)
        b.instructions = out
    return n


def _build_nc():
    nc = bass.Bass("TRN2", target_bir_lowering=False, debug=False)

    # x hi/lo interleaved: [:, :, dc, 0, :] = hi, [:, :, dc, 1, :] = lo
    xt8 = nc.dram_tensor("xt8", [NBLK, 128, NDC, 2, TB], FP8,
                         kind="ExternalInput").ap()
    # weights (lo, hi) order so a (x_hi, x_lo) rhs pairing yields cross terms
    wq8 = nc.dram_tensor("wq8", [128, NDC, 2, HD], FP8, kind="ExternalInput").ap()
    wk8 = nc.dram_tensor("wk8", [128, NDC, 2, HD], FP8, kind="ExternalInput").ap()
    wv8 = nc.dram_tensor("wv8", [128, NDC, 2, HD], FP8, kind="ExternalInput").ap()
    wot = nc.dram_tensor("wot", [HD, DIM], F16, kind="ExternalInput").ap()
    cosf = nc.dram_tensor("cosf", [128, TB], F16, kind="ExternalInput").ap()
    sinf = nc.dram_tensor("sinf", [128, TB], F16, kind="ExternalInput").ap()
    ident = nc.dram_tensor("ident", [128, 128], F16, kind="ExternalInput").ap()
    p64 = nc.dram_tensor("p64", [128, 128], F16, kind="ExternalInput").ap()
    ones = nc.dram_tensor("ones", [128, 1], F16, kind="ExternalInput").ap()
    # big store: token (blk, c*128 + p) lives at out0[blk, p, c]; the ragged
    # 16-token tail of each block goes to out1. Host reassembles.
    out0 = nc.dram_tensor("out0", [NBLK, 128, 3 * DIM], F16,
                          kind="ExternalOutput").ap()
    out1 = nc.dram_tensor("out1", [NBLK // 2, 2, 16, DIM], F16,
                          kind="ExternalOutput").ap()

    with _TileContextSplitDrain(nc) as tc:
        with (
            tc.tile_pool(name="singles", bufs=1) as singles,
            tc.tile_pool(name="xt", bufs=4) as xt_pool,
            # PSUM is 8 banks; four 2-buf pools keyed by use class so that
            # next-block projections only wait on this block's evacuations,
            # not on the whole attention chain.
            tc.tile_pool(name="qkv_ps", bufs=2, space="PSUM") as qkv_ps,
            tc.tile_pool(name="swsc_ps", bufs=2, space="PSUM") as swsc_ps,
            tc.tile_pool(name="misc_ps", bufs=2, space="PSUM") as misc_ps,
            tc.tile_pool(name="out_ps", bufs=2, space="PSUM") as out_ps,
            tc.tile_pool(name="ropetmp", bufs=6) as ropetmp,
            tc.tile_pool(name="heads", bufs=6) as heads,
            tc.tile_pool(name="attn_sb", bufs=6) as attn_sb_pool,
            tc.tile_pool(name="stats", bufs=12) as stats,
            tc.tile_pool(name="outsb", bufs=6) as outsb,
        ):
            # ---- PE p-state pre-warm: the first real matmul cannot start
            # until the weight/x DMA latency chain (~3.5us) resolves, and
            # the PE clock needs ~3us of continuous busy to reach 2.4GHz.
            # Chew through dummy matmuls on a memset tile meanwhile so real
            # work starts at full clock. ----
            warm = singles.tile([128, 256], F16, name="warm", tag="warm")
            nc.gpsimd.memset(warm, 0.0)
            for i in range(16):
                wps = out_ps.tile([128, 256], F32, name="warm_ps",
                                  tag="out_ps")
                nc.tensor.matmul(wps, lhsT=warm[:, 0:128], rhs=warm,
                                 start=True, stop=True)

            # ---- one-time loads (wk + first x chunks first: the k
            # projection is issued first and gates block 0) ----
            w_sb = {}
            t = singles.tile([128, NDC, 2, HD], FP8, name="wk8", tag="wk8")
            nc.sync.dma_start(out=t, in_=wk8)
            w_sb["k"] = t
            xt0_sb = xt_pool.tile([128, NDC, 2, TB], FP8, name="xt", tag="xt")
            for j in range(4):
                nc.sync.dma_start(out=xt0_sb[:, 2 * j:2 * j + 2, :, :],
                                  in_=xt8[0, :, 2 * j:2 * j + 2, :, :])
            for name, src in (("q", wq8), ("v", wv8)):
                t = singles.tile([128, NDC, 2, HD], FP8, name="w" + name,
                                 tag="w" + name)
                nc.sync.dma_start(out=t, in_=src)
                w_sb[name] = t
            p64_sb = singles.tile([128, 128], F16, name="p64", tag="p64")
            nc.sync.dma_start(out=p64_sb, in_=p64)
            cos_sb = singles.tile([128, TB], F16, name="cosf", tag="cosf")
            nc.sync.dma_start(out=cos_sb, in_=cosf)
            sin_sb = singles.tile([128, TB], F16, name="sinf", tag="sinf")
            nc.sync.dma_start(out=sin_sb, in_=sinf)
            id_sb = singles.tile([128, 128], F16, name="ident", tag="ident")
            nc.sync.dma_start(out=id_sb, in_=ident)
            ones_sb = singles.tile([128, 1], F16, name="ones", tag="ones")
            nc.sync.dma_start(out=ones_sb, in_=ones)
            wot_sb = singles.tile([HD, DIM], F16, name="wot", tag="wot")
            nc.sync.dma_start(out=wot_sb, in_=wot)

            for blk in range(NBLK):
                # ---- load x hi/lo for this block ----
                if blk == 0:
                    xt_sb = xt0_sb
                else:
                    xt_sb = xt_pool.tile([128, NDC, 2, TB], FP8, name="xt",
                                         tag="xt")
                    nc.sync.dma_start(out=xt_sb[:, 0:4, :, :],
                                      in_=xt8[blk, :, 0:4, :, :])
                    nc.sync.dma_start(out=xt_sb[:, 4:8, :, :],
                                      in_=xt8[blk, :, 4:8, :, :])

                # ---- QKV projections: fp8 DoubleRow, hi/lo compensated ----
                def proj(wname):
                    w = w_sb[wname]
                    ps = qkv_ps.tile([128, TB], F32, name="proj_ps",
                                     tag="proj_ps")
                    mms = []
                    for j in range(NDC // 2):   # hi @ hi over d-chunk pairs
                        mms.append((w[:, 2 * j:2 * j + 2, 1, :],
                                    xt_sb[:, 2 * j:2 * j + 2, 0, :]))
                    for dc in range(NDC):       # x_hi@w_lo + x_lo@w_hi
                        mms.append((w[:, dc, :, :], xt_sb[:, dc, :, :]))
                    for i, (lhs, rhs) in enumerate(mms):
                        nc.tensor.matmul(ps, lhsT=lhs, rhs=rhs,
                                         start=(i == 0),
                                         stop=(i == len(mms) - 1),
                                         perf_mode=DRMODE)
                    return ps

                # k first: its rope chain gates the score matmuls
                k_ps = proj("k")
                q_ps = proj("q")
                v_ps = proj("v")

                # ---- RoPE (de-interleaved rotate-half form) ----
                # swap(q)[p] = q[(p+64)%128] runs on the PE via a permutation
                # matmul (rhs must be SBUF, hence the ACT evacuation first,
                # which also removes the 512x fp8 prescale).
                def rope(ps, tag, fast):
                    # q (the late chain, gating scores) runs split per batch
                    # half with the final add on DVE right behind the u-mul
                    # (same in-order queue, no sem hop). k has timeline
                    # slack and keeps the cheap Pool path.
                    qsb = ropetmp.tile([128, TB], F16, name="pre_" + tag,
                                       tag="pre_" + tag)
                    sw_ps = swsc_ps.tile([128, TB], F32, name="swsc_ps",
                                         tag="swsc_ps")
                    c = ropetmp.tile([128, TB], F16, name="rope_c" + tag,
                                     tag="rope_c" + tag)
                    u = ropetmp.tile([128, TB], F16, name="rope_u" + tag,
                                     tag="rope_u" + tag)
                    h = heads.tile([128, TB], F16, name=tag, tag=tag)
                    sls = ([slice(i * S, (i + 1) * S) for i in range(BB)]
                           if fast else [slice(0, TB)])
                    # latency-critical chain: bias the scheduler to pick
                    # these over the previous block's throughput work
                    with tc.high_priority(offset=250):
                        for sl in sls:
                            nc.scalar.mul(qsb[:, sl], ps[:, sl], EVAC_SCALE)
                            nc.tensor.matmul(sw_ps[:, sl], lhsT=p64_sb,
                                             rhs=qsb[:, sl],
                                             start=True, stop=True)
                            nc.gpsimd.tensor_mul(c[:, sl], qsb[:, sl],
                                                 cos_sb[:, sl])
                            # sin table is sign-folded ([-sin; +sin]) so one
                            # add completes the rotation
                            nc.vector.tensor_mul(u[:, sl], sw_ps[:, sl],
                                                 sin_sb[:, sl])
                            if fast:
                                nc.vector.tensor_add(h[:, sl], c[:, sl],
                                                     u[:, sl])
                            else:
                                nc.gpsimd.tensor_add(h[:, sl], c[:, sl],
                                                     u[:, sl])
                    return h

                k_h = rope(k_ps, "k_h", fast=False)
                q_h = rope(q_ps, "q_h", fast=True)
                v_h = heads.tile([128, TB], F16, name="v_h", tag="v_h")
                with tc.high_priority(offset=250):
                    nc.scalar.mul(v_h, v_ps, EVAC_SCALE)

                # ---- scores [k, q] and exp ----
                est = []
                for kc, (k0, ksz) in enumerate(KCH):
                    sp = swsc_ps.tile([128, TB], F32, name="swsc_ps",
                                      tag="swsc_ps")
                    for i in range(BB):
                        nc.tensor.matmul(
                            sp[0:ksz, i * S:(i + 1) * S],
                            lhsT=k_h[:, i * S + k0: i * S + k0 + ksz],
                            rhs=q_h[:, i * S:(i + 1) * S],
                            start=True, stop=True,
                        )
                    e = attn_sb_pool.tile([128, TB], F16, name="exp_st",
                                          tag="exp_st")
                    if blk == NBLK - 1:
                        for i in range(BB):
                            nc.scalar.activation(
                                out=e[0:ksz, i * S:(i + 1) * S],
                                in_=sp[0:ksz, i * S:(i + 1) * S],
                                func=mybir.ActivationFunctionType.Exp,
                                scale=EXP_SCALE,
                            )
                    else:
                        nc.scalar.activation(
                            out=e[0:ksz, :], in_=sp[0:ksz, :],
                            func=mybir.ActivationFunctionType.Exp,
                            scale=EXP_SCALE,
                        )
                    est.append(e)

                # ---- softmax denominators: est^T @ 1 per token chunk ----
                # tail (16 tokens) goes to partition slot 64*(blk%2) so two
                # blocks' tails share one 128-partition group chunk later
                # (only bases 0/32/64 are legal; quadrant 3 is unusable).
                gslot = 64 * (blk % 2)
                if blk % 2 == 0:
                    grp_rec = stats.tile([128, 1], F32, name="grec",
                                         tag="grec")
                    nc.gpsimd.memset(grp_rec, 1.0)
                    grp_av = attn_sb_pool.tile([128, 128], F16, name="gav",
                                               tag="gav")
                    nc.gpsimd.memset(grp_av, 0.0)
                recips = {}
                sums_ps = misc_ps.tile([128, 8], F32, name="misc_ps",
                                       tag="misc_ps")
                for tc_i, (t0, tsz) in enumerate(TCH):
                    p0 = gslot if tc_i == 3 else 0
                    for kc, (k0, ksz) in enumerate(KCH):
                        nc.tensor.matmul(
                            sums_ps[p0:p0 + tsz, tc_i:tc_i + 1],
                            lhsT=est[kc][0:ksz, t0:t0 + tsz],
                            rhs=ones_sb[0:ksz, :],
                            start=(kc == 0), stop=(kc == len(KCH) - 1),
                        )
                    if tc_i == 3:
                        nc.vector.reciprocal(
                            grp_rec[p0:p0 + tsz, :],
                            sums_ps[p0:p0 + tsz, tc_i:tc_i + 1])
                    else:
                        rec = stats.tile([128, 1], F32, name=f"recip{tc_i}",
                                         tag=f"recip{tc_i}")
                        nc.vector.reciprocal(rec[0:tsz, :],
                                             sums_ps[0:tsz, tc_i:tc_i + 1])
                        recips[tc_i] = rec

                # ---- V -> seq-major via fp16 PE transpose, per batch ----
                vt_ps = misc_ps.tile([128, 512], F16, name="misc_ps",
                                     tag="misc_ps")
                vt_sbs = []
                for i in range(BB):
                    nc.tensor.transpose(
                        vt_ps[0:128, i * 256: i * 256 + 128],
                        v_h[:, i * S: i * S + 128], id_sb,
                    )
                    nc.tensor.transpose(
                        vt_ps[0:72, i * 256 + 128: i * 256 + 256],
                        v_h[:, i * S + 128: (i + 1) * S], id_sb,
                    )
                    vt_sb = attn_sb_pool.tile([128, 256], F16, name="vt_sb",
                                              tag="vt_sb")
                    nc.vector.tensor_copy(
                        vt_sb[0:128, 0:128],
                        vt_ps[0:128, i * 256: i * 256 + 128])
                    nc.vector.tensor_copy(
                        vt_sb[0:72, 128:256],
                        vt_ps[0:72, i * 256 + 128: i * 256 + 256])
                    vt_sbs.append(vt_sb)

                # ---- AV: attn_head[h, q] (unnormalized) ----
                av_ps = misc_ps.tile([128, TB], F32, name="misc_ps",
                                     tag="misc_ps")
                for i in range(BB):
                    nc.tensor.matmul(
                        av_ps[:, i * S:(i + 1) * S],
                        lhsT=vt_sbs[i][0:128, 0:128],
                        rhs=est[0][0:128, i * S:(i + 1) * S],
                        start=True, stop=False,
                    )
                    nc.tensor.matmul(
                        av_ps[:, i * S:(i + 1) * S],
                        lhsT=vt_sbs[i][0:72, 128:256],
                        rhs=est[1][0:72, i * S:(i + 1) * S],
                        start=False, stop=True,
                    )
                av_sb = attn_sb_pool.tile([128, TB], F16, name="av_sb",
                                          tag="av_sb")
                # two halves so the first outproj chunks start earlier;
                # the 16-token tail goes to this group's shared tile
                if blk == NBLK - 1:
                    nc.vector.tensor_copy(av_sb[:, 0:128], av_ps[:, 0:128])
                    nc.vector.tensor_copy(av_sb[:, 128:256],
                                          av_ps[:, 128:256])
                    nc.vector.tensor_copy(av_sb[:, 256:384],
                                          av_ps[:, 256:384])
                else:
                    nc.vector.tensor_copy(av_sb[:, 0:200], av_ps[:, 0:200])
                    nc.vector.tensor_copy(av_sb[:, 200:384],
                                          av_ps[:, 200:384])
                nc.vector.tensor_copy(grp_av[:, gslot:gslot + 16],
                                      av_ps[:, 384:TB])

                # ---- output projection + normalization + store ----
                # full 128-token chunks span the batch boundary; stores go
                # out per chunk so they pipeline with the evacuations. The
                # last block borrows the (by then idle) other PSUM pools for
                # a deeper outproj rotation, since there is no next-block
                # work left to hide the evacuation latency behind.
                last = blk == NBLK - 1
                osb = outsb.tile([128, 3 * DIM], F16, name="osb", tag="osb")
                for tc_i, (t0, tsz) in enumerate(TCH[:3]):
                    for dc in range(2):
                        pool, tag = ([(out_ps, "out_ps"), (misc_ps, "misc_ps"),
                                      (swsc_ps, "swsc_ps")][(tc_i * 2 + dc) % 3]
                                     if last else (out_ps, "out_ps"))
                        ops = pool.tile([128, 512], F32, name=tag, tag=tag)
                        nc.tensor.matmul(
                            ops[0:tsz, :],
                            lhsT=av_sb[:, t0:t0 + tsz],
                            rhs=wot_sb[:, dc * 512:(dc + 1) * 512],
                            start=True, stop=True,
                        )
                        dst = osb[0:tsz,
                                  tc_i * DIM + dc * 512:
                                  tc_i * DIM + (dc + 1) * 512]
                        if dc == 0:
                            nc.scalar.mul(dst, ops[0:tsz, :],
                                          recips[tc_i][0:tsz, :])
                        else:
                            nc.vector.tensor_scalar_mul(
                                dst, ops[0:tsz, :], recips[tc_i][0:tsz, :])
                    nc.sync.dma_start(
                        out=out0[blk, :, tc_i * DIM:(tc_i + 1) * DIM],
                        in_=osb[:, tc_i * DIM:(tc_i + 1) * DIM])

                # ---- grouped tail outproj every 4th block ----
                if blk % 2 == 1:
                    gosb = outsb.tile([128, DIM], F16, name="gosb",
                                      tag="gosb")
                    for dc in range(2):
                        ops = out_ps.tile([128, 512], F32, name="out_ps",
                                          tag="out_ps")
                        nc.tensor.matmul(
                            ops,
                            lhsT=grp_av,
                            rhs=wot_sb[:, dc * 512:(dc + 1) * 512],
                            start=True, stop=True,
                        )
                        dst = gosb[:, dc * 512:(dc + 1) * 512]
                        if dc == 0:
                            nc.scalar.mul(dst, ops, grp_rec)
                        else:
                            nc.vector.tensor_scalar_mul(dst, ops, grp_rec)
                    nc.sync.dma_start(out=out1[blk // 2, 0],
                                      in_=gosb[0:16, :])
                    nc.sync.dma_start(out=out1[blk // 2, 1],
                                      in_=gosb[64:80, :])
    _split_excess_waits(nc)
    return nc


_NC_CACHE = {}


def _get_nc():
    if "nc" not in _NC_CACHE:
        _NC_CACHE["nc"] = _build_nc()
    return _NC_CACHE["nc"]


def _hilo(a):
    hi = np.asarray(a, ml_dtypes.float8_e4m3fn)
    lo = np.asarray(a - hi.astype(np.float32), ml_dtypes.float8_e4m3fn)
    return hi, lo


def _host_prep(x, wq, wk, wv, wo):
    """Shared (non-x) device inputs + per-core x fp8 hi/lo shards."""
    perm = np.concatenate([np.arange(0, HD, 2), np.arange(1, HD, 2)])

    def wprep(w, permute):
        wp = (w[perm] if permute else w) * WS
        # layout [p, dc, 2, h]: row d of w.T at (p=d%128, dc=d//128);
        # index 2 is (lo, hi)
        wt = np.ascontiguousarray(
            wp.T.reshape(NDC, 128, HD).transpose(1, 0, 2))
        hi, lo = _hilo(wt)
        return np.ascontiguousarray(np.stack([lo, hi], axis=2))

    wq8 = wprep(wq, True)
    wk8 = wprep(wk, True)
    wv8 = wprep(wv, False)
    wot = np.ascontiguousarray(wo.T.astype(np.float16))

    inv_freq = 1.0 / BASE ** (np.arange(0, HD, 2, dtype=np.float64) / HD)
    ang = np.zeros((S, HD // 2), np.float64)
    ang[1:] = np.arange(S - 1, dtype=np.float64)[:, None] * inv_freq[None, :]
    cos_t = np.cos(ang).T  # [64, S]
    sin_t = np.sin(ang).T
    cosf = np.tile(np.concatenate([cos_t, cos_t], axis=0),
                   (1, BB)).astype(np.float16)
    # sign-folded: rotated = q*cosf + swap64(q)*sinf in one add
    sinf = np.tile(np.concatenate([-sin_t, sin_t], axis=0),
                   (1, BB)).astype(np.float16)

    shared = {
        "wq8": wq8, "wk8": wk8, "wv8": wv8, "wot": wot,
        "cosf": np.ascontiguousarray(cosf),
        "sinf": np.ascontiguousarray(sinf),
        "ident": np.eye(128, dtype=np.float16),
        "p64": np.ascontiguousarray(
            np.roll(np.eye(128, dtype=np.float16), 64, axis=1)),
        "ones": np.ones((128, 1), np.float16),
    }
    xs = x.reshape(N_CORES, NBLK, TB, NDC, 128) * XS
    # [core, blk, p, dc, t]
    xt = np.ascontiguousarray(xs.transpose(0, 1, 4, 3, 2))
    hi, lo = _hilo(xt)
    x8 = np.stack([hi, lo], axis=4)  # [core, blk, p, dc, 2, t]
    xts = [np.ascontiguousarray(x8[c]) for c in range(N_CORES)]
    return shared, xts


def kernel(x, wq, wk, wv, wo):
    x = np.asarray(x, np.float32)
    wq = np.asarray(wq, np.float32)
    wk = np.asarray(wk, np.float32)
    wv = np.asarray(wv, np.float32)
    wo = np.asarray(wo, np.float32)

    shared, xts = _host_prep(x, wq, wk, wv, wo)
    in_maps = [dict(shared, xt8=xts[c]) for c in range(N_CORES)]
    nc = _get_nc()
    res = run_bass_kernel_spmd(nc, in_maps, list(range(N_CORES)))
    full = np.empty((B * S, DIM), np.float32)
    for c in range(N_CORES):
        o0 = np.asarray(res.results[c]["out0"]).astype(np.float32)
        o1 = np.asarray(res.results[c]["out1"]).astype(np.float32)
        core = full[c * BS * S:(c + 1) * BS * S].reshape(NBLK, TB, DIM)
        # token (blk, cch*128 + p) came from out0[blk, p, cch]
        core[:, 0:384, :] = o0.reshape(NBLK, 128, 3, DIM).transpose(
            0, 2, 1, 3).reshape(NBLK, 384, DIM)
        # tail token (blk, 384+i) came from out1[blk//2, 64*(blk%2)+i]
        core[:, 384:400, :] = o1.reshape(NBLK, 16, DIM)
    return full.reshape(B, S, DIM)


# revision 65
# speedup vs baseline: 1.0286x; 1.0083x over previous
"""Trainium2 Bass kernel for single-head attention with RoPE.

Problem (per full input): x [256, 200, 1024], wq/wk/wv [128, 1024], wo [1024, 128]
  q/k/v = x @ w*.T ; RoPE on q,k (positions 1..S-1, class token 0 unrotated)
  out = softmax(q k^T / sqrt(128)) v @ wo.T

Strategy: data-parallel over batch across 8 NeuronCores (32 batches/core),
processed in 16 blocks of 2 batches (400 tokens). All I/O is half-width:
x ships as an error-compensated fp8 hi/lo pair, the output returns fp16.

Per block:
  - QKV projections run on the PE in fp8e4 DoubleRow mode (K=256 per pass).
    Plain fp8 is too coarse (~6% per-element), so operands are split
    x ~ x_hi + x_lo, w ~ w_hi + w_lo (both fp8; w pre-scaled by 64 and x by 8
    so the residuals stay in fp8's normal range) and three of the four
    products are accumulated in PSUM: hi*hi (4 DR matmuls over d-chunk pairs)
    plus the two cross terms, computed 2-at-a-time by pairing DoubleRow's two
    k-tiles as (x_hi, x_lo) against (w_lo, w_hi) per d-chunk (8 DR matmuls).
    The dropped lo*lo term is ~0.4%% of the result. The 1/512 prescale comes
    out in the PSUM->SBUF evacuation (free ACT constant scale).
  - RoPE in de-interleaved rotate-half form (wq/wk rows permuted host-side):
    the half-swap runs as a fp16 permutation matmul on the PE; the two
    multiplies and the add are spread over DVE and GPSIMD with fp16 tables.
  - scores [k, q] only (k on partitions): softmax denominators come from
    tiny ones-vector matmuls (est^T @ 1 -> [q,1] in PSUM) instead of a second
    transposed score pass; exp on ACT writes fp16 est straight to SBUF.
  - V is transposed to seq-major via fp16 PE transposes (fp16 PSUM out).
  - output projection per 128-token chunk (chunks span the two batches);
    softmax normalization (1/rowsum) folds into the PSUM->SBUF evacuation as
    a per-partition scale, which also converts to fp16 for the store. The
    ragged 16-token tail of each block is parked at partition slot 64*(blk%2)
    (only bases 0/32/64 are legal matmul output positions) and two blocks'
    tails share one outproj chunk, quartering the tail cost.
  - stores go out per chunk into a [blk, p, chunk] layout (plus the tail
    tensor); the host reassembles and upcasts. PSUM's 8 banks are split into
    four 2-buffer pools by use class so next-block projections only wait on
    this block's evacuations; the last block rotates its outproj through the
    idle pools to drain faster.
"""

import math

import numpy as np
import ml_dtypes

import concourse.bass as bass
import concourse.mybir as mybir
import concourse.tile as tile
from concourse.bass_utils import run_bass_kernel_spmd

B, S, DIM, HD = 256, 200, 1024, 128
BASE = 10000.0
N_CORES = 8
BS = B // N_CORES      # 32 batches per core
BB = 2                 # batches per block
TB = BB * S            # 400 tokens per block
NBLK = BS // BB        # 16 blocks per core
NDC = DIM // 128       # 8 contraction chunks
F32 = mybir.dt.float32
F16 = mybir.dt.float16
FP8 = mybir.dt.float8e4
DRMODE = mybir.MatmulPerfMode.DoubleRow
EXP_SCALE = 1.0 / math.sqrt(HD)
XS = 8.0               # x fp8 prescale
WS = 64.0              # weight fp8 prescale
EVAC_SCALE = 1.0 / (XS * WS)
# output-token chunks within a block (span the batch boundary; tokens are
# contiguous in [BS*S, DIM] so each chunk stores as one 2D DMA)
TCH = [(0, 128), (128, 128), (256, 128), (384, 16)]
# scores/est partition chunks over k positions within one batch
KCH = [(0, 128), (128, S - 128)]


class _TileContextSplitDrain(tile.TileContext):
    """Workaround: this walrus build rejects >2 sem-wait commands on the
    kernel-tail Drain. Emit each needed wait as its own instruction first."""

    def _drain_and_barrier(self, tick_clock, wait_clock):
        nc = self.nc
        fake = mybir.InstNoOp(
            name=nc.get_next_instruction_name(), ins=[], outs=[],
            engine=mybir.EngineType.SP,
        )
        wait_clock.add_sem_waits(
            fake, tile.ScopedClock({None: tick_clock.global_clock})
        )
        waits = list(fake.sync_info.on_wait) if fake.sync_info is not None else []
        assert self.sems is not None
        handles = {h.name: h for h in self.sems.allocated().values()}
        for w in waits:
            nc.sync.wait_ge(handles[w.ant_name], w.wait_value)
        nc.sync.drain()
        nc.all_engine_barrier()
        popped = nc._tile_sem_poison_stack.pop()
        assert popped is self._sem_poison
        nc.clear_and_free_semaphores(list(self.sems.allocated().values()))
        nc.all_engine_barrier()


def _split_excess_waits(nc):
    """This walrus build accepts 1 sem-wait per instruction (2 on
    EventSemaphore). Tile may attach more; hoist the excess onto standalone
    EventSemaphore instructions right before the owner (same engine, so
    in-order issue preserves the wait semantics)."""
    n = 0
    for b in nc.m.functions[0].blocks:
        insts = b.instructions
        out = []
        for i in insts:
            si = i.sync_info
            if si is not None and len(si.on_wait) > 1:
                keep = 2 if isinstance(i, mybir.InstEventSemaphore) else 1
                waits = list(si.on_wait)
                for w in waits[:-keep] if keep < len(waits) else []:
                    n += 1
                    out.append(mybir.InstEventSemaphore(
                        name=f"{i.name}-evw{n}", ins=[], outs=[],
                        engine=i.engine,
                        sync_info=mybir.SyncInfo(on_wait=[w], on_update=[]),
                    ))
                i.sync_info = mybir.SyncInfo(
                    on_wait=waits[-keep:], on_update=list(si.on_update)
                )
            out.append(i)
        b.instructions = out
    return n


def _build_nc():
    nc = bass.Bass("TRN2", target_bir_lowering=False, debug=False)

    # x hi/lo interleaved: [:, :, dc, 0, :] = hi, [:, :, dc, 1, :] = lo
    xt8 = nc.dram_tensor("xt8", [NBLK, 128, NDC, 2, TB], FP8,
                         kind="ExternalInput").ap()
    # weights (lo, hi) order so a (x_hi, x_lo) rhs pairing yields cross terms
    wq8 = nc.dram_tensor("wq8", [128, NDC, 2, HD], FP8, kind="ExternalInput").ap()
    wk8 = nc.dram_tensor("wk8", [128, NDC, 2, HD], FP8, kind="ExternalInput").ap()
    wv8 = nc.dram_tensor("wv8", [128, NDC, 2, HD], FP8, kind="ExternalInput").ap()
    wot = nc.dram_tensor("wot", [HD, DIM], F16, kind="ExternalInput").ap()
    cosf = nc.dram_tensor("cosf", [128, TB], F16, kind="ExternalInput").ap()
    sinf = nc.dram_tensor("sinf", [128, TB], F16, kind="ExternalInput").ap()
    ident = nc.dram_tensor("ident", [128, 128], F16, kind="ExternalInput").ap()
    p64 = nc.dram_tensor("p64", [128, 128], F16, kind="ExternalInput").ap()
    ones = nc.dram_tensor("ones", [128, 1], F16, kind="ExternalInput").ap()
    # big store: token (blk, c*128 + p) lives at out0[blk, p, c]; the ragged
    # 16-token tail of each block goes to out1. Host reassembles.
    out0 = nc.dram_tensor("out0", [NBLK, 128, 3 * DIM], F16,
                          kind="ExternalOutput").ap()
    out1 = nc.dram_tensor("out1", [NBLK // 2, 2, 16, DIM], F16,
                          kind="ExternalOutput").ap()

    with _TileContextSplitDrain(nc) as tc:
        with (
            tc.tile_pool(name="singles", bufs=1) as singles,
            tc.tile_pool(name="xt", bufs=4) as xt_pool,
            # PSUM is 8 banks; four 2-buf pools keyed by use class so that
            # next-block projections only wait on this block's evacuations,
            # not on the whole attention chain.
            tc.tile_pool(name="qkv_ps", bufs=2, space="PSUM") as qkv_ps,
            tc.tile_pool(name="swsc_ps", bufs=2, space="PSUM") as swsc_ps,
            tc.tile_pool(name="misc_ps", bufs=2, space="PSUM") as misc_ps,
            tc.tile_pool(name="out_ps", bufs=2, space="PSUM") as out_ps,
            tc.tile_pool(name="ropetmp", bufs=6) as ropetmp,
            tc.tile_pool(name="heads", bufs=6) as heads,
            tc.tile_pool(name="attn_sb", bufs=6) as attn_sb_pool,
            tc.tile_pool(name="stats", bufs=12) as stats,
            tc.tile_pool(name="outsb", bufs=6) as outsb,
        ):
            # ---- PE p-state pre-warm: the first real matmul cannot start
            # until the weight/x DMA latency chain (~3.5us) resolves, and
            # the PE clock needs ~3us of continuous busy to reach 2.4GHz.
            # Chew through dummy matmuls on a memset tile meanwhile so real
            # work starts at full clock. ----
            warm = singles.tile([128, 256], F16, name="warm", tag="warm")
            nc.gpsimd.memset(warm, 0.0)
            for i in range(16):
                wps = out_ps.tile([128, 256], F32, name="warm_ps",
                                  tag="out_ps")
                nc.tensor.matmul(wps, lhsT=warm[:, 0:128], rhs=warm,
                                 start=True, stop=True)

            # ---- one-time loads (wk + first x chunks first: the k
            # projection is issued first and gates block 0) ----
            w_sb = {}
            t = singles.tile([128, NDC, 2, HD], FP8, name="wk8", tag="wk8")
            nc.sync.dma_start(out=t, in_=wk8)
            w_sb["k"] = t
            xt0_sb = xt_pool.tile([128, NDC, 2, TB], FP8, name="xt", tag="xt")
            for j in range(4):
                nc.sync.dma_start(out=xt0_sb[:, 2 * j:2 * j + 2, :, :],
                                  in_=xt8[0, :, 2 * j:2 * j + 2, :, :])
            for name, src in (("q", wq8), ("v", wv8)):
                t = singles.tile([128, NDC, 2, HD], FP8, name="w" + name,
                                 tag="w" + name)
                nc.sync.dma_start(out=t, in_=src)
                w_sb[name] = t
            p64_sb = singles.tile([128, 128], F16, name="p64", tag="p64")
            nc.sync.dma_start(out=p64_sb, in_=p64)
            cos_sb = singles.tile([128, TB], F16, name="cosf", tag="cosf")
            nc.sync.dma_start(out=cos_sb, in_=cosf)
            sin_sb = singles.tile([128, TB], F16, name="sinf", tag="sinf")
            nc.sync.dma_start(out=sin_sb, in_=sinf)
            id_sb = singles.tile([128, 128], F16, name="ident", tag="ident")
            nc.sync.dma_start(out=id_sb, in_=ident)
            ones_sb = singles.tile([128, 1], F16, name="ones", tag="ones")
            nc.sync.dma_start(out=ones_sb, in_=ones)
            wot_sb = singles.tile([HD, DIM], F16, name="wot", tag="wot")
            nc.sync.dma_start(out=wot_sb, in_=wot)

            for blk in range(NBLK):
                # ---- load x hi/lo for this block ----
                if blk == 0:
                    xt_sb = xt0_sb
                else:
                    xt_sb = xt_pool.tile([128, NDC, 2, TB], FP8, name="xt",
                                         tag="xt")
                    nc.sync.dma_start(out=xt_sb[:, 0:4, :, :],
                                      in_=xt8[blk, :, 0:4, :, :])
                    nc.sync.dma_start(out=xt_sb[:, 4:8, :, :],
                                      in_=xt8[blk, :, 4:8, :, :])

                # ---- QKV projections: fp8 DoubleRow, hi/lo compensated ----
                def proj(wname):
                    w = w_sb[wname]
                    ps = qkv_ps.tile([128, TB], F32, name="proj_ps",
                                     tag="proj_ps")
                    mms = []
                    for j in range(NDC // 2):   # hi @ hi over d-chunk pairs
                        mms.append((w[:, 2 * j:2 * j + 2, 1, :],
                                    xt_sb[:, 2 * j:2 * j + 2, 0, :]))
                    for dc in range(NDC):       # x_hi@w_lo + x_lo@w_hi
                        mms.append((w[:, dc, :, :], xt_sb[:, dc, :, :]))
                    for i, (lhs, rhs) in enumerate(mms):
                        nc.tensor.matmul(ps, lhsT=lhs, rhs=rhs,
                                         start=(i == 0),
                                         stop=(i == len(mms) - 1),
                                         perf_mode=DRMODE)
                    return ps

                # k first: its rope chain gates the score matmuls
                k_ps = proj("k")
                q_ps = proj("q")
                v_ps = proj("v")

                # ---- RoPE (de-interleaved rotate-half form) ----
                # swap(q)[p] = q[(p+64)%128] runs on the PE via a permutation
                # matmul (rhs must be SBUF, hence the ACT evacuation first,
                # which also removes the 512x fp8 prescale).
                def rope(ps, tag, fast):
                    # q (the late chain, gating scores) runs split per batch
                    # half with the final add on DVE right behind the u-mul
                    # (same in-order queue, no sem hop). k has timeline
                    # slack and keeps the cheap Pool path.
                    qsb = ropetmp.tile([128, TB], F16, name="pre_" + tag,
                                       tag="pre_" + tag)
                    sw_ps = swsc_ps.tile([128, TB], F32, name="swsc_ps",
                                         tag="swsc_ps")
                    c = ropetmp.tile([128, TB], F16, name="rope_c" + tag,
                                     tag="rope_c" + tag)
                    u = ropetmp.tile([128, TB], F16, name="rope_u" + tag,
                                     tag="rope_u" + tag)
                    h = heads.tile([128, TB], F16, name=tag, tag=tag)
                    sls = ([slice(i * S, (i + 1) * S) for i in range(BB)]
                           if fast else [slice(0, TB)])
                    # latency-critical chain: bias the scheduler to pick
                    # these over the previous block's throughput work
                    with tc.high_priority(offset=150):
                        for sl in sls:
                            nc.scalar.mul(qsb[:, sl], ps[:, sl], EVAC_SCALE)
                            nc.tensor.matmul(sw_ps[:, sl], lhsT=p64_sb,
                                             rhs=qsb[:, sl],
                                             start=True, stop=True)
                            nc.gpsimd.tensor_mul(c[:, sl], qsb[:, sl],
                                                 cos_sb[:, sl])
                            # sin table is sign-folded ([-sin; +sin]) so one
                            # add completes the rotation
                            nc.vector.tensor_mul(u[:, sl], sw_ps[:, sl],
                                                 sin_sb[:, sl])
                            if fast:
                                nc.vector.tensor_add(h[:, sl], c[:, sl],
                                                     u[:, sl])
                            else:
                                nc.gpsimd.tensor_add(h[:, sl], c[:, sl],
                                                     u[:, sl])
                    return h

                k_h = rope(k_ps, "k_h", fast=False)
                q_h = rope(q_ps, "q_h", fast=True)
                v_h = heads.tile([128, TB], F16, name="v_h", tag="v_h")
                nc.scalar.mul(v_h, v_ps, EVAC_SCALE)

                # ---- scores [k, q] and exp ----
                est = []
                for kc, (k0, ksz) in enumerate(KCH):
                    sp = swsc_ps.tile([128, TB], F32, name="swsc_ps",
                                      tag="swsc_ps")
                    for i in range(BB):
                        nc.tensor.matmul(
                            sp[0:ksz, i * S:(i + 1) * S],
                            lhsT=k_h[:, i * S + k0: i * S + k0 + ksz],
                            rhs=q_h[:, i * S:(i + 1) * S],
                            start=True, stop=True,
                        )
                    e = attn_sb_pool.tile([128, TB], F16, name="exp_st",
                                          tag="exp_st")
                    if blk == NBLK - 1:
                        for i in range(BB):
                            nc.scalar.activation(
                                out=e[0:ksz, i * S:(i + 1) * S],
                                in_=sp[0:ksz, i * S:(i + 1) * S],
                                func=mybir.ActivationFunctionType.Exp,
                                scale=EXP_SCALE,
                            )
                    else:
                        nc.scalar.activation(
                            out=e[0:ksz, :], in_=sp[0:ksz, :],
                            func=mybir.ActivationFunctionType.Exp,
                            scale=EXP_SCALE,
                        )
                    est.append(e)

                # ---- softmax denominators: est^T @ 1 per token chunk ----
                # tail (16 tokens) goes to partition slot 64*(blk%2) so two
                # blocks' tails share one 128-partition group chunk later
                # (only bases 0/32/64 are legal; quadrant 3 is unusable).
                gslot = 64 * (blk % 2)
                if blk % 2 == 0:
                    grp_rec = stats.tile([128, 1], F32, name="grec",
                                         tag="grec")
                    nc.gpsimd.memset(grp_rec, 1.0)
                    grp_av = attn_sb_pool.tile([128, 128], F16, name="gav",
                                               tag="gav")
                    nc.gpsimd.memset(grp_av, 0.0)
                recips = {}
                sums_ps = misc_ps.tile([128, 8], F32, name="misc_ps",
                                       tag="misc_ps")
                for tc_i, (t0, tsz) in enumerate(TCH):
                    p0 = gslot if tc_i == 3 else 0
                    for kc, (k0, ksz) in enumerate(KCH):
                        nc.tensor.matmul(
                            sums_ps[p0:p0 + tsz, tc_i:tc_i + 1],
                            lhsT=est[kc][0:ksz, t0:t0 + tsz],
                            rhs=ones_sb[0:ksz, :],
                            start=(kc == 0), stop=(kc == len(KCH) - 1),
                        )
                    if tc_i == 3:
                        nc.vector.reciprocal(
                            grp_rec[p0:p0 + tsz, :],
                            sums_ps[p0:p0 + tsz, tc_i:tc_i + 1])
                    else:
                        rec = stats.tile([128, 1], F32, name=f"recip{tc_i}",
                                         tag=f"recip{tc_i}")
                        nc.vector.reciprocal(rec[0:tsz, :],
                                             sums_ps[0:tsz, tc_i:tc_i + 1])
                        recips[tc_i] = rec

                # ---- V -> seq-major via fp16 PE transpose, per batch ----
                vt_ps = misc_ps.tile([128, 512], F16, name="misc_ps",
                                     tag="misc_ps")
                vt_sbs = []
                for i in range(BB):
                    nc.tensor.transpose(
                        vt_ps[0:128, i * 256: i * 256 + 128],
                        v_h[:, i * S: i * S + 128], id_sb,
                    )
                    nc.tensor.transpose(
                        vt_ps[0:72, i * 256 + 128: i * 256 + 256],
                        v_h[:, i * S + 128: (i + 1) * S], id_sb,
                    )
                    vt_sb = attn_sb_pool.tile([128, 256], F16, name="vt_sb",
                                              tag="vt_sb")
                    nc.vector.tensor_copy(
                        vt_sb[0:128, 0:128],
                        vt_ps[0:128, i * 256: i * 256 + 128])
                    nc.vector.tensor_copy(
                        vt_sb[0:72, 128:256],
                        vt_ps[0:72, i * 256 + 128: i * 256 + 256])
                    vt_sbs.append(vt_sb)

                # ---- AV: attn_head[h, q] (unnormalized) ----
                av_ps = misc_ps.tile([128, TB], F32, name="misc_ps",
                                     tag="misc_ps")
                for i in range(BB):
                    nc.tensor.matmul(
                        av_ps[:, i * S:(i + 1) * S],
                        lhsT=vt_sbs[i][0:128, 0:128],
                        rhs=est[0][0:128, i * S:(i + 1) * S],
                        start=True, stop=False,
                    )
                    nc.tensor.matmul(
                        av_ps[:, i * S:(i + 1) * S],
                        lhsT=vt_sbs[i][0:72, 128:256],
                        rhs=est[1][0:72, i * S:(i + 1) * S],
                        start=False, stop=True,
                    )
                av_sb = attn_sb_pool.tile([128, TB], F16, name="av_sb",
                                          tag="av_sb")
                # two halves so the first outproj chunks start earlier;
                # the 16-token tail goes to this group's shared tile
                if blk == NBLK - 1:
                    nc.vector.tensor_copy(av_sb[:, 0:128], av_ps[:, 0:128])
                    nc.vector.tensor_copy(av_sb[:, 128:256],
                                          av_ps[:, 128:256])
                    nc.vector.tensor_copy(av_sb[:, 256:384],
                                          av_ps[:, 256:384])
                else:
                    nc.vector.tensor_copy(av_sb[:, 0:200], av_ps[:, 0:200])
                    nc.vector.tensor_copy(av_sb[:, 200:384],
                                          av_ps[:, 200:384])
                nc.vector.tensor_copy(grp_av[:, gslot:gslot + 16],
                                      av_ps[:, 384:TB])

                # ---- output projection + normalization + store ----
                # full 128-token chunks span the batch boundary; stores go
                # out per chunk so they pipeline with the evacuations. The
                # last block borrows the (by then idle) other PSUM pools for
                # a deeper outproj rotation, since there is no next-block
                # work left to hide the evacuation latency behind.
                last = blk == NBLK - 1
                osb = outsb.tile([128, 3 * DIM], F16, name="osb", tag="osb")
                for tc_i, (t0, tsz) in enumerate(TCH[:3]):
                    for dc in range(2):
                        pool, tag = ([(out_ps, "out_ps"), (misc_ps, "misc_ps"),
                                      (swsc_ps, "swsc_ps")][(tc_i * 2 + dc) % 3]
                                     if last else (out_ps, "out_ps"))
                        ops = pool.tile([128, 512], F32, name=tag, tag=tag)
                        nc.tensor.matmul(
                            ops[0:tsz, :],
                            lhsT=av_sb[:, t0:t0 + tsz],
                            rhs=wot_sb[:, dc * 512:(dc + 1) * 512],
                            start=True, stop=True,
                        )
                        dst = osb[0:tsz,
                                  tc_i * DIM + dc * 512:
                                  tc_i * DIM + (dc + 1) * 512]
                        # throughput work: sequence it behind the next
                        # block's latency-critical evacuations
                        with tc.high_priority(offset=-120):
                            if dc == 0:
                                nc.scalar.mul(dst, ops[0:tsz, :],
                                              recips[tc_i][0:tsz, :])
                            else:
                                nc.vector.tensor_scalar_mul(
                                    dst, ops[0:tsz, :],
                                    recips[tc_i][0:tsz, :])
                    nc.sync.dma_start(
                        out=out0[blk, :, tc_i * DIM:(tc_i + 1) * DIM],
                        in_=osb[:, tc_i * DIM:(tc_i + 1) * DIM])

                # ---- grouped tail outproj every 4th block ----
                if blk % 2 == 1:
                    gosb = outsb.tile([128, DIM], F16, name="gosb",
                                      tag="gosb")
                    for dc in range(2):
                        ops = out_ps.tile([128, 512], F32, name="out_ps",
                                          tag="out_ps")
                        nc.tensor.matmul(
                            ops,
                            lhsT=grp_av,
                            rhs=wot_sb[:, dc * 512:(dc + 1) * 512],
                            start=True, stop=True,
                        )
                        dst = gosb[:, dc * 512:(dc + 1) * 512]
                        with tc.high_priority(offset=-120):
                            if dc == 0:
                                nc.scalar.mul(dst, ops, grp_rec)
                            else:
                                nc.vector.tensor_scalar_mul(dst, ops,
                                                            grp_rec)
                    nc.sync.dma_start(out=out1[blk // 2, 0],
                                      in_=gosb[0:16, :])
                    nc.sync.dma_start(out=out1[blk // 2, 1],
                                      in_=gosb[64:80, :])
    _split_excess_waits(nc)
    return nc


_NC_CACHE = {}


def _get_nc():
    if "nc" not in _NC_CACHE:
        _NC_CACHE["nc"] = _build_nc()
    return _NC_CACHE["nc"]


def _hilo(a):
    hi = np.asarray(a, ml_dtypes.float8_e4m3fn)
    lo = np.asarray(a - hi.astype(np.float32), ml_dtypes.float8_e4m3fn)
    return hi, lo


def _host_prep(x, wq, wk, wv, wo):
    """Shared (non-x) device inputs + per-core x fp8 hi/lo shards."""
    perm = np.concatenate([np.arange(0, HD, 2), np.arange(1, HD, 2)])

    def wprep(w, permute):
        wp = (w[perm] if permute else w) * WS
        # layout [p, dc, 2, h]: row d of w.T at (p=d%128, dc=d//128);
        # index 2 is (lo, hi)
        wt = np.ascontiguousarray(
            wp.T.reshape(NDC, 128, HD).transpose(1, 0, 2))
        hi, lo = _hilo(wt)
        return np.ascontiguousarray(np.stack([lo, hi], axis=2))

    wq8 = wprep(wq, True)
    wk8 = wprep(wk, True)
    wv8 = wprep(wv, False)
    wot = np.ascontiguousarray(wo.T.astype(np.float16))

    inv_freq = 1.0 / BASE ** (np.arange(0, HD, 2, dtype=np.float64) / HD)
    ang = np.zeros((S, HD // 2), np.float64)
    ang[1:] = np.arange(S - 1, dtype=np.float64)[:, None] * inv_freq[None, :]
    cos_t = np.cos(ang).T  # [64, S]
    sin_t = np.sin(ang).T
    cosf = np.tile(np.concatenate([cos_t, cos_t], axis=0),
                   (1, BB)).astype(np.float16)
    # sign-folded: rotated = q*cosf + swap64(q)*sinf in one add
    sinf = np.tile(np.concatenate([-sin_t, sin_t], axis=0),
                   (1, BB)).astype(np.float16)

    shared = {
        "wq8": wq8, "wk8": wk8, "wv8": wv8, "wot": wot,
        "cosf": np.ascontiguousarray(cosf),
        "sinf": np.ascontiguousarray(sinf),
        "ident": np.eye(128, dtype=np.float16),
        "p64": np.ascontiguousarray(
            np.roll(np.eye(128, dtype=np.float16), 64, axis=1)),
        "ones": np.ones((128, 1), np.float16),
    }
    xs = x.reshape(N_CORES, NBLK, TB, NDC, 128) * XS
    # [core, blk, p, dc, t]
    xt = np.ascontiguousarray(xs.transpose(0, 1, 4, 3, 2))
    hi, lo = _hilo(xt)
    x8 = np.stack([hi, lo], axis=4)  # [core, blk, p, dc, 2, t]
    xts = [np.ascontiguousarray(x8[c]) for c in range(N_CORES)]
    return shared, xts


def kernel(x, wq, wk, wv, wo):
    x = np.asarray(x, np.float32)
    wq = np.asarray(wq, np.float32)
    wk = np.asarray(wk, np.float32)
    wv = np.asarray(wv, np.float32)
    wo = np.asarray(wo, np.float32)

    shared, xts = _host_prep(x, wq, wk, wv, wo)
    in_maps = [dict(shared, xt8=xts[c]) for c in range(N_CORES)]
    nc = _get_nc()
    res = run_bass_kernel_spmd(nc, in_maps, list(range(N_CORES)))
    full = np.empty((B * S, DIM), np.float32)
    for c in range(N_CORES):
        o0 = np.asarray(res.results[c]["out0"]).astype(np.float32)
        o1 = np.asarray(res.results[c]["out1"]).astype(np.float32)
        core = full[c * BS * S:(c + 1) * BS * S].reshape(NBLK, TB, DIM)
        # token (blk, cch*128 + p) came from out0[blk, p, cch]
        core[:, 0:384, :] = o0.reshape(NBLK, 128, 3, DIM).transpose(
            0, 2, 1, 3).reshape(NBLK, 384, DIM)
        # tail token (blk, 384+i) came from out1[blk//2, 64*(blk%2)+i]
        core[:, 384:400, :] = o1.reshape(NBLK, 16, DIM)
    return full.reshape(B, S, DIM)


# revision 66
# speedup vs baseline: 1.0359x; 1.0071x over previous
"""Trainium2 Bass kernel for single-head attention with RoPE.

Problem (per full input): x [256, 200, 1024], wq/wk/wv [128, 1024], wo [1024, 128]
  q/k/v = x @ w*.T ; RoPE on q,k (positions 1..S-1, class token 0 unrotated)
  out = softmax(q k^T / sqrt(128)) v @ wo.T

Strategy: data-parallel over batch across 8 NeuronCores (32 batches/core),
processed in 16 blocks of 2 batches (400 tokens). All I/O is half-width:
x ships as an error-compensated fp8 hi/lo pair, the output returns fp16.

Per block:
  - QKV projections run on the PE in fp8e4 DoubleRow mode (K=256 per pass).
    Plain fp8 is too coarse (~6% per-element), so operands are split
    x ~ x_hi + x_lo, w ~ w_hi + w_lo (both fp8; w pre-scaled by 64 and x by 8
    so the residuals stay in fp8's normal range) and three of the four
    products are accumulated in PSUM: hi*hi (4 DR matmuls over d-chunk pairs)
    plus the two cross terms, computed 2-at-a-time by pairing DoubleRow's two
    k-tiles as (x_hi, x_lo) against (w_lo, w_hi) per d-chunk (8 DR matmuls).
    The dropped lo*lo term is ~0.4%% of the result. The 1/512 prescale comes
    out in the PSUM->SBUF evacuation (free ACT constant scale).
  - RoPE in de-interleaved rotate-half form (wq/wk rows permuted host-side):
    the half-swap runs as a fp16 permutation matmul on the PE; the two
    multiplies and the add are spread over DVE and GPSIMD with fp16 tables.
  - scores [k, q] only (k on partitions): softmax denominators come from
    tiny ones-vector matmuls (est^T @ 1 -> [q,1] in PSUM) instead of a second
    transposed score pass; exp on ACT writes fp16 est straight to SBUF.
  - V is transposed to seq-major via fp16 PE transposes (fp16 PSUM out).
  - output projection per 128-token chunk (chunks span the two batches);
    softmax normalization (1/rowsum) folds into the PSUM->SBUF evacuation as
    a per-partition scale, which also converts to fp16 for the store. The
    ragged 16-token tail of each block is parked at partition slot 64*(blk%2)
    (only bases 0/32/64 are legal matmul output positions) and two blocks'
    tails share one outproj chunk, quartering the tail cost.
  - stores go out per chunk into a [blk, p, chunk] layout (plus the tail
    tensor); the host reassembles and upcasts. PSUM's 8 banks are split into
    four 2-buffer pools by use class so next-block projections only wait on
    this block's evacuations; the last block rotates its outproj through the
    idle pools to drain faster.
"""

import math

import numpy as np
import ml_dtypes

import concourse.bass as bass
import concourse.mybir as mybir
import concourse.tile as tile
from concourse.bass_utils import run_bass_kernel_spmd

B, S, DIM, HD = 256, 200, 1024, 128
BASE = 10000.0
N_CORES = 8
BS = B // N_CORES      # 32 batches per core
BB = 2                 # batches per block
TB = BB * S            # 400 tokens per block
NBLK = BS // BB        # 16 blocks per core
NDC = DIM // 128       # 8 contraction chunks
F32 = mybir.dt.float32
F16 = mybir.dt.float16
FP8 = mybir.dt.float8e4
DRMODE = mybir.MatmulPerfMode.DoubleRow
EXP_SCALE = 1.0 / math.sqrt(HD)
XS = 8.0               # x fp8 prescale
WS = 64.0              # weight fp8 prescale
EVAC_SCALE = 1.0 / (XS * WS)
# output-token chunks within a block (span the batch boundary; tokens are
# contiguous in [BS*S, DIM] so each chunk stores as one 2D DMA)
TCH = [(0, 128), (128, 128), (256, 128), (384, 16)]
# scores/est partition chunks over k positions within one batch
KCH = [(0, 128), (128, S - 128)]


class _TileContextSplitDrain(tile.TileContext):
    """Workaround: this walrus build rejects >2 sem-wait commands on the
    kernel-tail Drain. Emit each needed wait as its own instruction first."""

    def _drain_and_barrier(self, tick_clock, wait_clock):
        nc = self.nc
        fake = mybir.InstNoOp(
            name=nc.get_next_instruction_name(), ins=[], outs=[],
            engine=mybir.EngineType.SP,
        )
        wait_clock.add_sem_waits(
            fake, tile.ScopedClock({None: tick_clock.global_clock})
        )
        waits = list(fake.sync_info.on_wait) if fake.sync_info is not None else []
        assert self.sems is not None
        handles = {h.name: h for h in self.sems.allocated().values()}
        for w in waits:
            nc.sync.wait_ge(handles[w.ant_name], w.wait_value)
        nc.sync.drain()
        nc.all_engine_barrier()
        popped = nc._tile_sem_poison_stack.pop()
        assert popped is self._sem_poison
        nc.clear_and_free_semaphores(list(self.sems.allocated().values()))
        nc.all_engine_barrier()


def _split_excess_waits(nc):
    """This walrus build accepts 1 sem-wait per instruction (2 on
    EventSemaphore). Tile may attach more; hoist the excess onto standalone
    EventSemaphore instructions right before the owner (same engine, so
    in-order issue preserves the wait semantics)."""
    n = 0
    for b in nc.m.functions[0].blocks:
        insts = b.instructions
        out = []
        for i in insts:
            si = i.sync_info
            if si is not None and len(si.on_wait) > 1:
                keep = 2 if isinstance(i, mybir.InstEventSemaphore) else 1
                waits = list(si.on_wait)
                for w in waits[:-keep] if keep < len(waits) else []:
                    n += 1
                    out.append(mybir.InstEventSemaphore(
                        name=f"{i.name}-evw{n}", ins=[], outs=[],
                        engine=i.engine,
                        sync_info=mybir.SyncInfo(on_wait=[w], on_update=[]),
                    ))
                i.sync_info = mybir.SyncInfo(
                    on_wait=waits[-keep:], on_update=list(si.on_update)
                )
            out.append(i)
        b.instructions = out
    return n


def _build_nc():
    nc = bass.Bass("TRN2", target_bir_lowering=False, debug=False)

    # x hi/lo interleaved: [:, :, dc, 0, :] = hi, [:, :, dc, 1, :] = lo
    xt8 = nc.dram_tensor("xt8", [NBLK, 128, NDC, 2, TB], FP8,
                         kind="ExternalInput").ap()
    # weights (lo, hi) order so a (x_hi, x_lo) rhs pairing yields cross terms
    wq8 = nc.dram_tensor("wq8", [128, NDC, 2, HD], FP8, kind="ExternalInput").ap()
    wk8 = nc.dram_tensor("wk8", [128, NDC, 2, HD], FP8, kind="ExternalInput").ap()
    wv8 = nc.dram_tensor("wv8", [128, NDC, 2, HD], FP8, kind="ExternalInput").ap()
    wot = nc.dram_tensor("wot", [HD, DIM], F16, kind="ExternalInput").ap()
    cosf = nc.dram_tensor("cosf", [128, TB], F16, kind="ExternalInput").ap()
    sinf = nc.dram_tensor("sinf", [128, TB], F16, kind="ExternalInput").ap()
    ident = nc.dram_tensor("ident", [128, 128], F16, kind="ExternalInput").ap()
    p64 = nc.dram_tensor("p64", [128, 128], F16, kind="ExternalInput").ap()
    ones = nc.dram_tensor("ones", [128, 1], F16, kind="ExternalInput").ap()
    # big store: token (blk, c*128 + p) lives at out0[blk, p, c]; the ragged
    # 16-token tail of each block goes to out1. Host reassembles.
    out0 = nc.dram_tensor("out0", [NBLK, 128, 3 * DIM], F16,
                          kind="ExternalOutput").ap()
    out1 = nc.dram_tensor("out1", [NBLK // 2, 2, 16, DIM], F16,
                          kind="ExternalOutput").ap()

    with _TileContextSplitDrain(nc) as tc:
        with (
            tc.tile_pool(name="singles", bufs=1) as singles,
            tc.tile_pool(name="xt", bufs=4) as xt_pool,
            # PSUM is 8 banks; four 2-buf pools keyed by use class so that
            # next-block projections only wait on this block's evacuations,
            # not on the whole attention chain.
            tc.tile_pool(name="qkv_ps", bufs=2, space="PSUM") as qkv_ps,
            tc.tile_pool(name="swsc_ps", bufs=2, space="PSUM") as swsc_ps,
            tc.tile_pool(name="misc_ps", bufs=2, space="PSUM") as misc_ps,
            tc.tile_pool(name="out_ps", bufs=2, space="PSUM") as out_ps,
            tc.tile_pool(name="ropetmp", bufs=6) as ropetmp,
            tc.tile_pool(name="heads", bufs=6) as heads,
            tc.tile_pool(name="attn_sb", bufs=6) as attn_sb_pool,
            tc.tile_pool(name="stats", bufs=12) as stats,
            tc.tile_pool(name="outsb", bufs=6) as outsb,
        ):
            # ---- PE p-state pre-warm: the first real matmul cannot start
            # until the weight/x DMA latency chain (~3.5us) resolves, and
            # the PE clock needs ~3us of continuous busy to reach 2.4GHz.
            # Chew through dummy matmuls on a memset tile meanwhile so real
            # work starts at full clock. ----
            warm = singles.tile([128, 256], F16, name="warm", tag="warm")
            nc.gpsimd.memset(warm, 0.0)
            for i in range(16):
                wps = out_ps.tile([128, 256], F32, name="warm_ps",
                                  tag="out_ps")
                nc.tensor.matmul(wps, lhsT=warm[:, 0:128], rhs=warm,
                                 start=True, stop=True)

            # ---- one-time loads (wk + first x chunks first: the k
            # projection is issued first and gates block 0) ----
            w_sb = {}
            t = singles.tile([128, NDC, 2, HD], FP8, name="wk8", tag="wk8")
            nc.sync.dma_start(out=t, in_=wk8)
            w_sb["k"] = t
            xt0_sb = xt_pool.tile([128, NDC, 2, TB], FP8, name="xt", tag="xt")
            for j in range(4):
                nc.sync.dma_start(out=xt0_sb[:, 2 * j:2 * j + 2, :, :],
                                  in_=xt8[0, :, 2 * j:2 * j + 2, :, :])
            for name, src in (("q", wq8), ("v", wv8)):
                t = singles.tile([128, NDC, 2, HD], FP8, name="w" + name,
                                 tag="w" + name)
                nc.sync.dma_start(out=t, in_=src)
                w_sb[name] = t
            p64_sb = singles.tile([128, 128], F16, name="p64", tag="p64")
            nc.sync.dma_start(out=p64_sb, in_=p64)
            cos_sb = singles.tile([128, TB], F16, name="cosf", tag="cosf")
            nc.sync.dma_start(out=cos_sb, in_=cosf)
            sin_sb = singles.tile([128, TB], F16, name="sinf", tag="sinf")
            nc.sync.dma_start(out=sin_sb, in_=sinf)
            id_sb = singles.tile([128, 128], F16, name="ident", tag="ident")
            nc.sync.dma_start(out=id_sb, in_=ident)
            ones_sb = singles.tile([128, 1], F16, name="ones", tag="ones")
            nc.sync.dma_start(out=ones_sb, in_=ones)
            wot_sb = singles.tile([HD, DIM], F16, name="wot", tag="wot")
            nc.sync.dma_start(out=wot_sb, in_=wot)

            for blk in range(NBLK):
                # ---- load x hi/lo for this block ----
                if blk == 0:
                    xt_sb = xt0_sb
                else:
                    xt_sb = xt_pool.tile([128, NDC, 2, TB], FP8, name="xt",
                                         tag="xt")
                    nc.sync.dma_start(out=xt_sb[:, 0:4, :, :],
                                      in_=xt8[blk, :, 0:4, :, :])
                    nc.sync.dma_start(out=xt_sb[:, 4:8, :, :],
                                      in_=xt8[blk, :, 4:8, :, :])

                # ---- QKV projections: fp8 DoubleRow, hi/lo compensated ----
                def proj(wname):
                    w = w_sb[wname]
                    ps = qkv_ps.tile([128, TB], F32, name="proj_ps",
                                     tag="proj_ps")
                    mms = []
                    for j in range(NDC // 2):   # hi @ hi over d-chunk pairs
                        mms.append((w[:, 2 * j:2 * j + 2, 1, :],
                                    xt_sb[:, 2 * j:2 * j + 2, 0, :]))
                    for dc in range(NDC):       # x_hi@w_lo + x_lo@w_hi
                        mms.append((w[:, dc, :, :], xt_sb[:, dc, :, :]))
                    for i, (lhs, rhs) in enumerate(mms):
                        nc.tensor.matmul(ps, lhsT=lhs, rhs=rhs,
                                         start=(i == 0),
                                         stop=(i == len(mms) - 1),
                                         perf_mode=DRMODE)
                    return ps

                # k first: its rope chain gates the score matmuls
                k_ps = proj("k")
                q_ps = proj("q")
                v_ps = proj("v")

                # ---- RoPE (de-interleaved rotate-half form) ----
                # swap(q)[p] = q[(p+64)%128] runs on the PE via a permutation
                # matmul (rhs must be SBUF, hence the ACT evacuation first,
                # which also removes the 512x fp8 prescale).
                def rope(ps, tag, fast):
                    # q (the late chain, gating scores) runs split per batch
                    # half with the final add on DVE right behind the u-mul
                    # (same in-order queue, no sem hop). k has timeline
                    # slack and keeps the cheap Pool path.
                    qsb = ropetmp.tile([128, TB], F16, name="pre_" + tag,
                                       tag="pre_" + tag)
                    sw_ps = swsc_ps.tile([128, TB], F32, name="swsc_ps",
                                         tag="swsc_ps")
                    c = ropetmp.tile([128, TB], F16, name="rope_c" + tag,
                                     tag="rope_c" + tag)
                    u = ropetmp.tile([128, TB], F16, name="rope_u" + tag,
                                     tag="rope_u" + tag)
                    h = heads.tile([128, TB], F16, name=tag, tag=tag)
                    sls = ([slice(i * S, (i + 1) * S) for i in range(BB)]
                           if fast else [slice(0, TB)])
                    # latency-critical chain: bias the scheduler to pick
                    # these over the previous block's throughput work
                    with tc.high_priority(offset=100):
                        for sl in sls:
                            nc.scalar.mul(qsb[:, sl], ps[:, sl], EVAC_SCALE)
                            nc.tensor.matmul(sw_ps[:, sl], lhsT=p64_sb,
                                             rhs=qsb[:, sl],
                                             start=True, stop=True)
                            nc.gpsimd.tensor_mul(c[:, sl], qsb[:, sl],
                                                 cos_sb[:, sl])
                            # sin table is sign-folded ([-sin; +sin]) so one
                            # add completes the rotation
                            nc.vector.tensor_mul(u[:, sl], sw_ps[:, sl],
                                                 sin_sb[:, sl])
                            if fast:
                                nc.vector.tensor_add(h[:, sl], c[:, sl],
                                                     u[:, sl])
                            else:
                                nc.gpsimd.tensor_add(h[:, sl], c[:, sl],
                                                     u[:, sl])
                    return h

                k_h = rope(k_ps, "k_h", fast=False)
                q_h = rope(q_ps, "q_h", fast=True)
                v_h = heads.tile([128, TB], F16, name="v_h", tag="v_h")
                nc.scalar.mul(v_h, v_ps, EVAC_SCALE)

                # ---- scores [k, q] and exp ----
                est = []
                for kc, (k0, ksz) in enumerate(KCH):
                    sp = swsc_ps.tile([128, TB], F32, name="swsc_ps",
                                      tag="swsc_ps")
                    for i in range(BB):
                        nc.tensor.matmul(
                            sp[0:ksz, i * S:(i + 1) * S],
                            lhsT=k_h[:, i * S + k0: i * S + k0 + ksz],
                            rhs=q_h[:, i * S:(i + 1) * S],
                            start=True, stop=True,
                        )
                    e = attn_sb_pool.tile([128, TB], F16, name="exp_st",
                                          tag="exp_st")
                    if blk == NBLK - 1:
                        for i in range(BB):
                            nc.scalar.activation(
                                out=e[0:ksz, i * S:(i + 1) * S],
                                in_=sp[0:ksz, i * S:(i + 1) * S],
                                func=mybir.ActivationFunctionType.Exp,
                                scale=EXP_SCALE,
                            )
                    else:
                        nc.scalar.activation(
                            out=e[0:ksz, :], in_=sp[0:ksz, :],
                            func=mybir.ActivationFunctionType.Exp,
                            scale=EXP_SCALE,
                        )
                    est.append(e)

                # ---- softmax denominators: est^T @ 1 per token chunk ----
                # tail (16 tokens) goes to partition slot 64*(blk%2) so two
                # blocks' tails share one 128-partition group chunk later
                # (only bases 0/32/64 are legal; quadrant 3 is unusable).
                gslot = 64 * (blk % 2)
                if blk % 2 == 0:
                    grp_rec = stats.tile([128, 1], F32, name="grec",
                                         tag="grec")
                    nc.gpsimd.memset(grp_rec, 1.0)
                    grp_av = attn_sb_pool.tile([128, 128], F16, name="gav",
                                               tag="gav")
                    nc.gpsimd.memset(grp_av, 0.0)
                recips = {}
                sums_ps = misc_ps.tile([128, 8], F32, name="misc_ps",
                                       tag="misc_ps")
                for tc_i, (t0, tsz) in enumerate(TCH):
                    p0 = gslot if tc_i == 3 else 0
                    for kc, (k0, ksz) in enumerate(KCH):
                        nc.tensor.matmul(
                            sums_ps[p0:p0 + tsz, tc_i:tc_i + 1],
                            lhsT=est[kc][0:ksz, t0:t0 + tsz],
                            rhs=ones_sb[0:ksz, :],
                            start=(kc == 0), stop=(kc == len(KCH) - 1),
                        )
                    if tc_i == 3:
                        nc.vector.reciprocal(
                            grp_rec[p0:p0 + tsz, :],
                            sums_ps[p0:p0 + tsz, tc_i:tc_i + 1])
                    else:
                        rec = stats.tile([128, 1], F32, name=f"recip{tc_i}",
                                         tag=f"recip{tc_i}")
                        nc.vector.reciprocal(rec[0:tsz, :],
                                             sums_ps[0:tsz, tc_i:tc_i + 1])
                        recips[tc_i] = rec

                # ---- V -> seq-major via fp16 PE transpose, per batch ----
                vt_ps = misc_ps.tile([128, 512], F16, name="misc_ps",
                                     tag="misc_ps")
                vt_sbs = []
                for i in range(BB):
                    nc.tensor.transpose(
                        vt_ps[0:128, i * 256: i * 256 + 128],
                        v_h[:, i * S: i * S + 128], id_sb,
                    )
                    nc.tensor.transpose(
                        vt_ps[0:72, i * 256 + 128: i * 256 + 256],
                        v_h[:, i * S + 128: (i + 1) * S], id_sb,
                    )
                    vt_sb = attn_sb_pool.tile([128, 256], F16, name="vt_sb",
                                              tag="vt_sb")
                    nc.vector.tensor_copy(
                        vt_sb[0:128, 0:128],
                        vt_ps[0:128, i * 256: i * 256 + 128])
                    nc.vector.tensor_copy(
                        vt_sb[0:72, 128:256],
                        vt_ps[0:72, i * 256 + 128: i * 256 + 256])
                    vt_sbs.append(vt_sb)

                # ---- AV: attn_head[h, q] (unnormalized) ----
                av_ps = misc_ps.tile([128, TB], F32, name="misc_ps",
                                     tag="misc_ps")
                for i in range(BB):
                    nc.tensor.matmul(
                        av_ps[:, i * S:(i + 1) * S],
                        lhsT=vt_sbs[i][0:128, 0:128],
                        rhs=est[0][0:128, i * S:(i + 1) * S],
                        start=True, stop=False,
                    )
                    nc.tensor.matmul(
                        av_ps[:, i * S:(i + 1) * S],
                        lhsT=vt_sbs[i][0:72, 128:256],
                        rhs=est[1][0:72, i * S:(i + 1) * S],
                        start=False, stop=True,
                    )
                av_sb = attn_sb_pool.tile([128, TB], F16, name="av_sb",
                                          tag="av_sb")
                # two halves so the first outproj chunks start earlier;
                # the 16-token tail goes to this group's shared tile
                if blk == NBLK - 1:
                    nc.vector.tensor_copy(av_sb[:, 0:128], av_ps[:, 0:128])
                    nc.vector.tensor_copy(av_sb[:, 128:256],
                                          av_ps[:, 128:256])
                    nc.vector.tensor_copy(av_sb[:, 256:384],
                                          av_ps[:, 256:384])
                else:
                    nc.vector.tensor_copy(av_sb[:, 0:200], av_ps[:, 0:200])
                    nc.vector.tensor_copy(av_sb[:, 200:384],
                                          av_ps[:, 200:384])
                nc.vector.tensor_copy(grp_av[:, gslot:gslot + 16],
                                      av_ps[:, 384:TB])

                # ---- output projection + normalization + store ----
                # full 128-token chunks span the batch boundary; stores go
                # out per chunk so they pipeline with the evacuations. The
                # last block borrows the (by then idle) other PSUM pools for
                # a deeper outproj rotation, since there is no next-block
                # work left to hide the evacuation latency behind.
                last = blk == NBLK - 1
                osb = outsb.tile([128, 3 * DIM], F16, name="osb", tag="osb")
                for tc_i, (t0, tsz) in enumerate(TCH[:3]):
                    for dc in range(2):
                        pool, tag = ([(out_ps, "out_ps"), (misc_ps, "misc_ps"),
                                      (swsc_ps, "swsc_ps")][(tc_i * 2 + dc) % 3]
                                     if last else (out_ps, "out_ps"))
                        ops = pool.tile([128, 512], F32, name=tag, tag=tag)
                        nc.tensor.matmul(
                            ops[0:tsz, :],
                            lhsT=av_sb[:, t0:t0 + tsz],
                            rhs=wot_sb[:, dc * 512:(dc + 1) * 512],
                            start=True, stop=True,
                        )
                        dst = osb[0:tsz,
                                  tc_i * DIM + dc * 512:
                                  tc_i * DIM + (dc + 1) * 512]
                        # throughput work: sequence it behind the next
                        # block's latency-critical evacuations
                        with tc.high_priority(offset=-120):
                            if dc == 0:
                                nc.scalar.mul(dst, ops[0:tsz, :],
                                              recips[tc_i][0:tsz, :])
                            else:
                                nc.vector.tensor_scalar_mul(
                                    dst, ops[0:tsz, :],
                                    recips[tc_i][0:tsz, :])
                    nc.sync.dma_start(
                        out=out0[blk, :, tc_i * DIM:(tc_i + 1) * DIM],
                        in_=osb[:, tc_i * DIM:(tc_i + 1) * DIM])

                # ---- grouped tail outproj every 4th block ----
                if blk % 2 == 1:
                    gosb = outsb.tile([128, DIM], F16, name="gosb",
                                      tag="gosb")
                    for dc in range(2):
                        ops = out_ps.tile([128, 512], F32, name="out_ps",
                                          tag="out_ps")
                        nc.tensor.matmul(
                            ops,
                            lhsT=grp_av,
                            rhs=wot_sb[:, dc * 512:(dc + 1) * 512],
                            start=True, stop=True,
                        )
                        dst = gosb[:, dc * 512:(dc + 1) * 512]
                        with tc.high_priority(offset=-120):
                            if dc == 0:
                                nc.scalar.mul(dst, ops, grp_rec)
                            else:
                                nc.vector.tensor_scalar_mul(dst, ops,
                                                            grp_rec)
                    nc.sync.dma_start(out=out1[blk // 2, 0],
                                      in_=gosb[0:16, :])
                    nc.sync.dma_start(out=out1[blk // 2, 1],
                                      in_=gosb[64:80, :])
    _split_excess_waits(nc)
    return nc


_NC_CACHE = {}


def _get_nc():
    if "nc" not in _NC_CACHE:
        _NC_CACHE["nc"] = _build_nc()
    return _NC_CACHE["nc"]


def _hilo(a):
    hi = np.asarray(a, ml_dtypes.float8_e4m3fn)
    lo = np.asarray(a - hi.astype(np.float32), ml_dtypes.float8_e4m3fn)
    return hi, lo


def _host_prep(x, wq, wk, wv, wo):
    """Shared (non-x) device inputs + per-core x fp8 hi/lo shards."""
    perm = np.concatenate([np.arange(0, HD, 2), np.arange(1, HD, 2)])

    def wprep(w, permute):
        wp = (w[perm] if permute else w) * WS
        # layout [p, dc, 2, h]: row d of w.T at (p=d%128, dc=d//128);
        # index 2 is (lo, hi)
        wt = np.ascontiguousarray(
            wp.T.reshape(NDC, 128, HD).transpose(1, 0, 2))
        hi, lo = _hilo(wt)
        return np.ascontiguousarray(np.stack([lo, hi], axis=2))

    wq8 = wprep(wq, True)
    wk8 = wprep(wk, True)
    wv8 = wprep(wv, False)
    wot = np.ascontiguousarray(wo.T.astype(np.float16))

    inv_freq = 1.0 / BASE ** (np.arange(0, HD, 2, dtype=np.float64) / HD)
    ang = np.zeros((S, HD // 2), np.float64)
    ang[1:] = np.arange(S - 1, dtype=np.float64)[:, None] * inv_freq[None, :]
    cos_t = np.cos(ang).T  # [64, S]
    sin_t = np.sin(ang).T
    cosf = np.tile(np.concatenate([cos_t, cos_t], axis=0),
                   (1, BB)).astype(np.float16)
    # sign-folded: rotated = q*cosf + swap64(q)*sinf in one add
    sinf = np.tile(np.concatenate([-sin_t, sin_t], axis=0),
                   (1, BB)).astype(np.float16)

    shared = {
        "wq8": wq8, "wk8": wk8, "wv8": wv8, "wot": wot,
        "cosf": np.ascontiguousarray(cosf),
        "sinf": np.ascontiguousarray(sinf),
        "ident": np.eye(128, dtype=np.float16),
        "p64": np.ascontiguousarray(
            np.roll(np.eye(128, dtype=np.float16), 64, axis=1)),
        "ones": np.ones((128, 1), np.float16),
    }
    xs = x.reshape(N_CORES, NBLK, TB, NDC, 128) * XS
    # [core, blk, p, dc, t]
    xt = np.ascontiguousarray(xs.transpose(0, 1, 4, 3, 2))
    hi, lo = _hilo(xt)
    x8 = np.stack([hi, lo], axis=4)  # [core, blk, p, dc, 2, t]
    xts = [np.ascontiguousarray(x8[c]) for c in range(N_CORES)]
    return shared, xts


def kernel(x, wq, wk, wv, wo):
    x = np.asarray(x, np.float32)
    wq = np.asarray(wq, np.float32)
    wk = np.asarray(wk, np.float32)
    wv = np.asarray(wv, np.float32)
    wo = np.asarray(wo, np.float32)

    shared, xts = _host_prep(x, wq, wk, wv, wo)
    in_maps = [dict(shared, xt8=xts[c]) for c in range(N_CORES)]
    nc = _get_nc()
    res = run_bass_kernel_spmd(nc, in_maps, list(range(N_CORES)))
    full = np.empty((B * S, DIM), np.float32)
    for c in range(N_CORES):
        o0 = np.asarray(res.results[c]["out0"]).astype(np.float32)
        o1 = np.asarray(res.results[c]["out1"]).astype(np.float32)
        core = full[c * BS * S:(c + 1) * BS * S].reshape(NBLK, TB, DIM)
        # token (blk, cch*128 + p) came from out0[blk, p, cch]
        core[:, 0:384, :] = o0.reshape(NBLK, 128, 3, DIM).transpose(
            0, 2, 1, 3).reshape(NBLK, 384, DIM)
        # tail token (blk, 384+i) came from out1[blk//2, 64*(blk%2)+i]
        core[:, 384:400, :] = o1.reshape(NBLK, 16, DIM)
    return full.reshape(B, S, DIM)
